# revision 44
# baseline (speedup 1.0000x reference)
"""Bass/Trainium2 kernel for nn_BiCRFModel: 2-layer BiLSTM + dense + CRF NLL.

Strategy (8-core pure data parallelism, 32 sequences/core):
  - Gate-input projections (x @ Wx + b) computed as per-row-chunk pre-GEMMs
    whose emission is INTERLEAVED with the LSTM step loop (chunk ci, then
    steps 4ci..4ci+3), so the PE fills its idle slots with pre-GEMM work,
    stays continuously busy (full p-state clock), and no separate pre-GEMM
    wall-time exists.  Pre-GEMM results go PSUM -> DRAM directly.
  - LSTM recurrence in "orientation A": batch(+both directions) in the
    partition dim (64 rows), gates in the free dim, gate column order
    [f, i, j, o].  Recurrent matmuls use hT as the stationary operand,
    f32r dtype, accumulating into two 512-wide PSUM banks (bank0 = f,i;
    bank1 = j,o) so activations can start after half the matmuls.
  - The per-step gate input xw is injected into PSUM via an identity
    matmul (f32r, off the h critical path) instead of a DVE add.
  - Backward direction = global time flip + per-step state masking
    (c,h *= [t < len]), which reproduces tf.reverse_sequence semantics
    exactly without any per-sequence gather.
  - Per-step PE transposes maintain hT and build the transposed layer
    output X{1,2}T in HBM for the next layer / dense layer.
  - CRF forward recurrence via a [32, 81] logsumexp (alpha_i + trans_ij),
    per-step validity masking; gold path scores via host-built one-hot /
    pair-count tensors contracted on device against logits / trans.
Output: per-core sum of NLL over its 32 sequences; host sums and /256.
"""

import contextlib

import numpy as np

B, T, E, H, K = 256, 256, 300, 256, 9
N_CORES = 8

_CACHE = {}


# ---------------------------------------------------------------- wait split
def _split_excess_waits(nc, max_waits=1):
    """This walrus build allows only 1 sync wait per instruction.  Hoist
    excess waits onto InstEventSemaphore carriers inserted just before the
    instruction (same engine -> same program order -> identical blocking)."""
    import bass_rust
    import concourse.mybir as mybir

    n_split = 0
    for fn in nc.m.functions:
        for bb in fn.blocks:
            insts = list(bb.instructions)
            out = []
            changed = False
            for ins in insts:
                si = getattr(ins, "sync_info", None)
                waits = list(si.on_wait) if si is not None and si.on_wait else []
                if len(waits) > max_waits:
                    keep = waits[:max_waits]
                    rest = waits[max_waits:]
                    for ci in range(0, len(rest), max_waits):
                        nop = mybir.InstEventSemaphore(
                            name=f"{ins.name}-waitsplit-{ci}", ins=[], outs=[]
                        )
                        nop.engine = ins.engine
                        nop.bass_nofuse = True
                        nop.sync_info = bass_rust.SyncInfo(
                            on_wait=list(rest[ci : ci + max_waits]), on_update=[]
                        )
                        out.append(nop)
                    si.on_wait = keep
                    n_split += 1
                    changed = True
                out.append(ins)
            if changed:
                bb.instructions[:] = out
    return n_split


# ---------------------------------------------------------------- builder
def build_nc(cfg, split=True):
    import concourse.bass as bass
    import concourse.mybir as mybir
    from concourse import tile

    f32 = mybir.dt.float32
    f32r = mybir.dt.float32r
    AF = mybir.ActivationFunctionType
    OP = mybir.AluOpType
    AX = mybir.AxisListType

    Tn = cfg["T"]
    BL = cfg["BL"]
    En = cfg["E"]
    Hn = cfg["H"]
    Kn = cfg["K"]
    EP = -(-En // 128) * 128          # padded input feat
    G4 = 4 * Hn                        # gate width
    HP = 2 * Hn                        # concat feat
    B2 = 2 * BL                        # fw+bw packed batch
    NKE = EP // 128
    NKH = Hn // 128
    NKX = HP // 128
    ROWS = Tn * BL
    NCH = ROWS // 128                  # row chunks
    TPC = 128 // BL                    # timesteps per chunk
    NB = G4 // 512                     # psum n-slices
    K2 = Kn * Kn
    NKH2 = 2 * NKH

    nc = bass.Bass("TRN2", num_devices=cfg["n_cores"])

    embT = nc.dram_tensor("embT", [EP, ROWS], f32r, kind="ExternalInput")
    m2_d = nc.dram_tensor("m2", [B2, Tn], f32, kind="ExternalInput")
    oh_d = nc.dram_tensor("oh", [ROWS, Kn], f32, kind="ExternalInput")
    c81_d = nc.dram_tensor("c81t", [K2, BL], f32, kind="ExternalInput")
    sel_d = nc.dram_tensor("sel", [128, BL], f32, kind="ExternalInput")
    id_d = nc.dram_tensor("identt", [128, BL], f32, kind="ExternalInput")
    idf_d = nc.dram_tensor("idf", [128, TPC * B2], f32r, kind="ExternalInput")
    idb_d = nc.dram_tensor("idb", [128, TPC * B2], f32r, kind="ExternalInput")
    tr81_d = nc.dram_tensor("tr81", [B2, K2], f32, kind="ExternalInput")
    mcrf_d = nc.dram_tensor("mcrf", [B2, Tn // 2 + 1], f32, kind="ExternalInput")
    tf_d = nc.dram_tensor("transflat", [K2, 1], f32, kind="ExternalInput")
    dw_d = nc.dram_tensor("dwc", [128, NKX * Kn], f32r, kind="ExternalInput")
    db_d = nc.dram_tensor("db", [1, Kn], f32r, kind="ExternalInput")
    on_d = nc.dram_tensor("ones1", [1, 128], f32r, kind="ExternalInput")
    wx_d, wh_d, bias_d = {}, {}, {}
    for l in (0, 1):
        nk = NKE if l == 0 else NKX
        for d in ("f", "b"):
            wx_d[(l, d)] = nc.dram_tensor(f"wx{l}{d}", [128, nk * G4], f32r, kind="ExternalInput")
            bias_d[(l, d)] = nc.dram_tensor(f"bias{l}{d}", [1, G4], f32r, kind="ExternalInput")
        wh_d[l] = nc.dram_tensor(f"wh{l}", [128, NKH2 * G4], f32r, kind="ExternalInput")
    out_d = nc.dram_tensor("out", [1, 1], f32, kind="ExternalOutput")

    with tile.TileContext(nc) as tc, contextlib.ExitStack() as ctx:
        cp = ctx.enter_context(tc.tile_pool(name="const", bufs=1))
        gp = ctx.enter_context(tc.tile_pool(name="work", bufs=2))
        sp = ctx.enter_context(tc.tile_pool(name="step", bufs=2))
        pp = ctx.enter_context(tc.tile_pool(name="psum", bufs=2, space="PSUM"))
        dp = ctx.enter_context(tc.tile_pool(name="dram", bufs=1, space="DRAM"))
        crf = ctx.enter_context(tc.tile_pool(name="crf", bufs=2))

        def cload(name, dram, shape, dt=f32):
            t = cp.tile(shape, dt, name=name, tag=name)
            nc.sync.dma_start(t[:], dram[:, :])
            return t

        m2s = cload("m2s", m2_d, [B2, Tn])
        c81s = cload("c81s", c81_d, [K2, BL])
        sels = cload("sels", sel_d, [128, BL])
        ids = cload("ids", id_d, [128, BL])
        idfs = cload("idfs", idf_d, [128, TPC * B2], f32r)
        idbs = cload("idbs", idb_d, [128, TPC * B2], f32r)
        tr81s = cload("tr81s", tr81_d, [B2, K2])
        mcrfs = cload("mcrfs", mcrf_d, [B2, Tn // 2 + 1])
        tfs = cload("tfs", tf_d, [K2, 1])
        wxs, whs, bss = {}, {}, {}
        for l in (0, 1):
            nk = NKE if l == 0 else NKX
            for d in ("f", "b"):
                wxs[(l, d)] = cload(f"wxs{l}{d}", wx_d[(l, d)], [128, nk * G4], f32r)
                bss[(l, d)] = cload(f"bss{l}{d}", bias_d[(l, d)], [1, G4], f32r)
            whs[l] = cload(f"whs{l}", wh_d[l], [128, NKH2 * G4], f32r)
        ones1 = cload("ones1s", on_d, [1, 128], f32r)
        onesb = cp.tile([BL, 1], f32, name="onesb", tag="onesb")
        nc.vector.memset(onesb[:], 1.0)

        x1t = dp.tile([HP, ROWS], f32r, name="x1t", tag="x1t")
        x2t = dp.tile([HP, ROWS], f32r, name="x2t", tag="x2t")
        lgd = dp.tile([ROWS, Kn], f32, name="lgd", tag="lgd")

        # ------------- pre-GEMM chunk: zs{f,b} = srcT.T @ Wx + b, kept in SBUF
        # zs rows are the 128 (4 timesteps x 32 batch) rows of the chunk; the
        # per-step identity matmuls read 32-row slices directly.
        def pre_gemm_chunk(l, src, nk, ci):
            out = {}
            for d in ("f", "b"):
                c = ci if d == "f" else NCH - 1 - ci
                xt = gp.tile([128, nk * 128], f32r, name=f"pgx{l}{d}{ci}", tag="pgx", bufs=3)
                nc.sync.dma_start(
                    xt[:].rearrange("p (k m) -> p k m", k=nk),
                    src[:, 128 * c : 128 * (c + 1)].rearrange("(k p) m -> p k m", k=nk),
                )
                zs = gp.tile([128, G4], f32r, name=f"pgs{l}{d}{ci}", tag=f"zs{d}", bufs=3)
                pbanks = (
                    (pp.tile([128, 512], f32, name=f"pgz{l}{d}{ci}_0", tag="zp0"), 0, 512),
                    (pp.tile([128, Hn], f32, name=f"pgzj{l}{d}{ci}", tag="zpj", bufs=1), 512, Hn),
                    (pp.tile([128, Hn], f32, name=f"pgzo{l}{d}{ci}", tag="zpo", bufs=1), 768, Hn),
                )
                for zpb, n0, nw in pbanks:
                    # layer 0 rides the bias on embT's ones-row (row E); layer 1
                    # needs an explicit rank-1 bias matmul.
                    if l != 0:
                        nc.tensor.matmul(
                            zpb[:], ones1[:], bss[(l, d)][:, n0 : n0 + nw],
                            start=True, stop=False,
                        )
                    for k in range(nk):
                        nc.tensor.matmul(
                            zpb[:],
                            xt[:, 128 * k : 128 * (k + 1)],
                            wxs[(l, d)][:, k * G4 + n0 : k * G4 + n0 + nw],
                            start=(l == 0 and k == 0),
                            stop=(k == nk - 1),
                        )
                for idx, (zpb, n0, nw) in enumerate(pbanks):
                    if idx == 0:
                        nc.scalar.copy(zs[:, n0 : n0 + nw], zpb[:])
                    else:
                        nc.vector.tensor_copy(zs[:, n0 : n0 + nw], zpb[:])
                out[d] = zs
            return out

        # ------------- one LSTM step (both dirs packed in 64 partitions)
        # Recurrent matmul uses a block-diagonal stationary operand so both
        # directions land in one M=64 base-0 PSUM write (f32r cannot write
        # PSUM at a partition offset): lhsT k-chunks 0..NKH-1 hold hT_fw in
        # cols 0:BL (rest zero), chunks NKH..2NKH-1 hold hT_bw in cols BL:2BL.
        def hT_dst(hTbig, di):
            return hTbig[:, di * NKH * B2 : (di + 1) * NKH * B2].rearrange(
                "p (c x) -> p c x", c=NKH
            )[:, :, di * BL : (di + 1) * BL]

        def lstm_step(l, s, c_prev, hTbig, xout, zsd):
            tfw, tbw = s, Tn - 1 - s
            j = s % TPC            # row band of zs['f'] for this step
            jb = TPC - 1 - j       # row band of zs['b'] (time-reversed chunk)

            # PSUM: bank0 = [f, i] (512), bankJ = [j] (256), bankO = [o] (256).
            # PE order: bank0's matmuls first (sigmoid f,i is the longest ACT
            # op), then bankJ (tanh j gates the c-chain), then bankO.
            zp0 = pp.tile([B2, 512], f32, name=f"slz{l}_{s}_0", tag="zp0")
            zpj = pp.tile([B2, Hn], f32, name=f"slzj{l}_{s}", tag="zpj", bufs=1)
            zpo = pp.tile([B2, Hn], f32, name=f"slzo{l}_{s}", tag="zpo", bufs=1)
            banks = ((zp0, 0, 512), (zpj, 512, 256), (zpo, 768, 256))
            for zpb, n0, nw in banks:
                nc.tensor.matmul(
                    zpb[:], idfs[:, B2 * j : B2 * (j + 1)],
                    zsd["f"][:, n0 : n0 + nw],
                    start=True, stop=False,
                )
                nc.tensor.matmul(
                    zpb[:], idbs[:, B2 * jb : B2 * (jb + 1)],
                    zsd["b"][:, n0 : n0 + nw],
                    start=False, stop=False,
                )
            for zpb, n0, nw in banks:
                for k in range(NKH2):
                    nc.tensor.matmul(
                        zpb[:],
                        hTbig[:, B2 * k : B2 * (k + 1)],
                        whs[l][:, k * G4 + n0 : k * G4 + n0 + nw],
                        start=False,
                        stop=(k == NKH2 - 1),
                    )

            # gate order [f, i | j | o]
            sfi = sp.tile([B2, 2 * Hn], f32, name=f"sfi{l}_{s}", tag="sfi")
            nc.scalar.activation(sfi[:], zp0[:], AF.Sigmoid)
            g = sp.tile([B2, Hn], f32, name=f"g{l}_{s}", tag="g")
            nc.scalar.activation(g[:], zpj[:], AF.Tanh)
            so = sp.tile([B2, Hn], f32, name=f"so{l}_{s}", tag="so")
            nc.scalar.activation(so[:], zpo[:], AF.Sigmoid)

            mcol = m2s[:, s : s + 1]
            t2 = sp.tile([B2, Hn], f32, name=f"t2{l}_{s}", tag="t2")
            nc.vector.scalar_tensor_tensor(
                t2[:], sfi[:, 0:Hn], mcol, c_prev[:], OP.mult, OP.mult
            )
            # the tail runs as two feature-half chains so tanh/h/transpose
            # pipeline between ACT, DVE and PE
            HH = Hn // 2
            c_new = sp.tile([B2, Hn], f32, name=f"c{l}_{s}", tag="cst", bufs=3)
            t1 = sp.tile([B2, Hn], f32, name=f"t1{l}_{s}", tag="t1")
            th = sp.tile([B2, Hn], f32, name=f"th{l}_{s}", tag="th")
            h = sp.tile([B2, Hn], f32, name=f"h{l}_{s}", tag="h")
            tps = []
            for di in range(2):
                tp = pp.tile(
                    [128, NKH * BL], f32, name=f"tp{di}_{l}_{s}",
                    tag=f"tp{di}", bufs=1,
                )
                tps.append(tp)
            for q in range(2):
                ql = slice(HH * q, HH * (q + 1))
                nc.vector.scalar_tensor_tensor(
                    t1[:, ql], sfi[:, Hn + HH * q : Hn + HH * (q + 1)], mcol,
                    g[:, ql], OP.mult, OP.mult,
                )
                nc.vector.tensor_tensor(c_new[:, ql], t1[:, ql], t2[:, ql], op=OP.add)
                nc.scalar.activation(th[:, ql], c_new[:, ql], AF.Tanh)
                nc.vector.scalar_tensor_tensor(
                    h[:, ql], so[:, ql], mcol, th[:, ql], OP.mult, OP.mult
                )
                # feature half q == hT k-chunk q: transpose as soon as ready
                k = q
                for di in range(2):
                    po = BL * di
                    nc.tensor.matmul(
                        tps[di][:, 32 * k : 32 * k + 32],
                        h[po : po + BL, 128 * k : 128 * (k + 1)],
                        ids[po : po + BL, 0:BL],
                        is_transpose=True,
                    )
                # copy each k-chunk into hTbig immediately; the next step's
                # k-chunk matmuls unblock per chunk (subtile deps)
                nc.scalar.copy(
                    hT_dst(hTbig, 0)[:, k : k + 1, :],
                    tps[0][:, 32 * k : 32 * k + 32].rearrange(
                        "p (c x) -> p c x", c=1
                    ),
                )
                nc.vector.tensor_copy(
                    hT_dst(hTbig, 1)[:, k : k + 1, :],
                    tps[1][:, 32 * k : 32 * k + 32].rearrange(
                        "p (c x) -> p c x", c=1
                    ),
                )
            for di, tdst in ((0, tfw), (1, tbw)):
                nc.gpsimd.dma_start(
                    xout[
                        Hn * di : Hn * (di + 1), BL * tdst : BL * (tdst + 1)
                    ].rearrange("(k p) b -> p k b", k=NKH),
                    hT_dst(hTbig, di),
                )
            return c_new

        # ------------- a full BiLSTM layer: pre-GEMM interleaved with steps
        def lstm_layer(l, src, nk, xout):
            z0 = sp.tile([128, NKH2 * B2], f32, name=f"z0_{l}", tag="z0")
            nc.vector.memset(z0[:], 0.0)
            hTbig = sp.tile([128, NKH2 * B2], f32r, name=f"hTbig{l}", tag="hTbig", bufs=1)
            nc.scalar.copy(hTbig[:], z0[:])
            c_prev = sp.tile([B2, Hn], f32, name=f"cinit{l}", tag="cst", bufs=3)
            nc.vector.memset(c_prev[:], 0.0)
            nsteps = min(Tn, cfg.get("nsteps", Tn))
            for ci in range(NCH):
                zsd = pre_gemm_chunk(l, src, nk, ci)
                for s in range(TPC * ci, min(TPC * (ci + 1), nsteps)):
                    c_prev = lstm_step(l, s, c_prev, hTbig, xout, zsd)
                if TPC * (ci + 1) >= nsteps:
                    break

        def logits_and_crf():
            # ---------------- logits (64 chunks of [128, K]) + unary accumulation
            lg = []
            dws32 = cp.tile([128, NKX * Kn], f32, name="dws32", tag="dws32")
            nc.sync.dma_start(dws32[:], dw_d[:, :].bitcast(f32))
            dbs32 = cp.tile([1, Kn], f32, name="dbs32", tag="dbs32")
            nc.sync.dma_start(dbs32[:], db_d[:, :].bitcast(f32))
            on32 = cp.tile([1, 128], f32, name="on32", tag="on32")
            nc.sync.dma_start(on32[:], on_d[:, :].bitcast(f32))
            usum = cp.tile([128, NCH], f32, name="usum", tag="usum")
            ohall = cp.tile([128, NCH * Kn], f32, name="ohall", tag="ohall")
            nc.sync.dma_start(
                ohall[:].rearrange("p (c k) -> p c k", c=NCH),
                oh_d[:, :].rearrange("(c p) k -> p c k", c=NCH),
            )
            for c in range(NCH):
                lp = pp.tile([128, Kn], f32, name=f"lp{c}", tag="psmall")
                nc.tensor.matmul(lp[:], on32[:], dbs32[:], start=True, stop=False)
                xt = gp.tile([128, NKX * 128], f32, name=f"lgx{c}", tag="lgx", bufs=4)
                nc.sync.dma_start(
                    xt[:].rearrange("p (k m) -> p k m", k=NKX),
                    x2t[:, 128 * c : 128 * (c + 1)].bitcast(f32).rearrange(
                        "(k p) m -> p k m", k=NKX
                    ),
                )
                for k in range(NKX):
                    nc.tensor.matmul(
                        lp[:],
                        xt[:, 128 * k : 128 * (k + 1)],
                        dws32[:, Kn * k : Kn * (k + 1)],
                        start=False,
                        stop=(k == NKX - 1),
                    )
                lgc = cp.tile([128, Kn], f32, name=f"lg{c}", tag=f"lg{c}")
                nc.vector.tensor_copy(lgc[:], lp[:])
                lg.append(lgc)
                nc.sync.dma_start(lgd[128 * c : 128 * (c + 1), :], lgc[:])
                scr = gp.tile([128, Kn], f32, name=f"ohscr{c}", tag="ohscr")
                nc.vector.scalar_tensor_tensor(
                    scr[:], lgc[:], 1.0, ohall[:, Kn * c : Kn * (c + 1)],
                    OP.mult, OP.mult,
                    accum_out=usum[:, c : c + 1],
                )

            # ---------------- gold-path scores
            up = pp.tile([BL, NCH], f32, name="up", tag="psmall")
            nc.tensor.matmul(up[:], sels[:], usum[:], start=True, stop=True)
            unary = cp.tile([BL, 1], f32, name="unary", tag="unary")
            nc.vector.reduce_sum(unary[:], up[:], axis=AX.X)
            bp = pp.tile([BL, 1], f32, name="bp", tag="psmall")
            nc.tensor.matmul(bp[:], c81s[:], tfs[:], start=True, stop=True)
            binry = cp.tile([BL, 1], f32, name="binry", tag="binry")
            nc.scalar.copy(binry[:], bp[:])

            # ---------------- CRF forward/backward split recurrence
            # Rows 0:BL run the forward alpha over t=0..M; rows BL:2BL run the
            # backward beta over t=T-1..M (M = T/2).  128 packed iterations,
            # then logZ = lse(alpha_M + beta_M).  Instead of a per-iteration
            # max-subtraction, exp uses a running -mx bias renormalized every
            # RENORM iterations (f32 exp headroom covers the drift).
            M = Tn // 2
            RENORM = 8
            # lgpost: rows 0:BL = logits blocks t=0..M (alpha post-add);
            #         rows BL:  = 0
            # lgpre:  rows BL:2BL = logits blocks t=0..255 (beta pre-add,
            #         indexed at 256-r); rows 0:BL = 0
            lgpost = cp.tile([B2, (M + 1) * Kn], f32, name="lgpost", tag="lgpost")
            nc.vector.memset(lgpost[:], 0.0)
            nc.sync.dma_start(
                lgpost[0:BL, :].rearrange("b (t k) -> b t k", k=Kn),
                lgd[0 : (M + 1) * BL, :].rearrange("(t b) k -> b t k", b=BL),
            )
            lgpre = cp.tile([B2, Tn * Kn], f32, name="lgpre", tag="lgpre")
            nc.vector.memset(lgpre[0:BL, :], 0.0)
            nc.sync.dma_start(
                lgpre[BL:B2, :].rearrange("b (t k) -> b t k", k=Kn),
                lgd[:, :].rearrange("(t b) k -> b t k", b=BL),
            )
            S = crf.tile([B2, Kn], f32, name="S0", tag="S")
            nc.vector.memset(S[BL:B2, :], 0.0)
            nc.vector.tensor_copy(S[0:BL, :], lgpost[0:BL, 0:Kn])
            mx = None
            for r in range(1, M + 1):
                if (r - 1) % RENORM == 0:
                    mx = crf.tile([B2, 1], f32, name=f"mx{r}", tag="mx")
                    nc.vector.reduce_max(mx[:], S[:], axis=AX.X)
                    nmx = crf.tile([B2, 1], f32, name=f"nmx{r}", tag="nmx")
                    nc.vector.tensor_scalar_mul(nmx[:], mx[:], -1.0)
                pre = crf.tile([B2, Kn], f32, name=f"pre{r}", tag="pre")
                nc.vector.tensor_tensor(
                    pre[:], S[:], lgpre[:, Kn * (Tn - r) : Kn * (Tn - r + 1)],
                    op=OP.add,
                )
                a81 = crf.tile([B2, K2], f32, name=f"a81_{r}", tag="a81")
                nc.vector.tensor_tensor(
                    a81[:].rearrange("p (j i) -> p j i", i=Kn),
                    pre[:].unsqueeze(1).broadcast_to([B2, Kn, Kn]),
                    tr81s[:].rearrange("p (j i) -> p j i", i=Kn),
                    op=OP.add,
                )
                e81 = crf.tile([B2, K2], f32, name=f"e81_{r}", tag="e81")
                nc.scalar.activation(e81[:], a81[:], AF.Exp, bias=nmx[:, 0:1])
                s9 = crf.tile([B2, Kn], f32, name=f"s9_{r}", tag="s9")
                nc.vector.reduce_sum(
                    s9[:], e81[:].rearrange("p (j i) -> p j i", i=Kn), axis=AX.X
                )
                lgs = crf.tile([B2, Kn], f32, name=f"lgs{r}", tag="lgs")
                nc.scalar.activation(lgs[:], s9[:], AF.Ln)
                cand = crf.tile([B2, Kn], f32, name=f"cand{r}", tag="cand")
                nc.vector.scalar_tensor_tensor(
                    cand[:], lgs[:], nmx[:, 0:1], lgpost[:, Kn * r : Kn * (r + 1)],
                    OP.subtract, OP.add,
                )
                dd = crf.tile([B2, Kn], f32, name=f"dd{r}", tag="dd")
                nc.vector.tensor_tensor(dd[:], cand[:], S[:], op=OP.subtract)
                snew = crf.tile([B2, Kn], f32, name=f"S{r}", tag="S")
                nc.vector.scalar_tensor_tensor(
                    snew[:], dd[:], mcrfs[:, r : r + 1], S[:], OP.mult, OP.add
                )
                S = snew

            # ---------------- logZ = lse(alpha_M + beta_M), nll, partial sum
            bet = crf.tile([BL, Kn], f32, name="bet", tag="bet")
            nc.vector.tensor_copy(bet[:], S[BL:B2, :])
            z9 = crf.tile([BL, Kn], f32, name="z9", tag="z9")
            nc.vector.tensor_tensor(z9[:], S[0:BL, :], bet[:], op=OP.add)
            mxf = crf.tile([BL, 1], f32, name="mxf", tag="mxf")
            nc.vector.reduce_max(mxf[:], z9[:], axis=AX.X)
            nmxf = crf.tile([BL, 1], f32, name="nmxf", tag="nmxf")
            nc.vector.tensor_scalar_mul(nmxf[:], mxf[:], -1.0)
            ef = crf.tile([BL, Kn], f32, name="ef", tag="ef")
            se = crf.tile([BL, 1], f32, name="se", tag="se")
            nc.scalar.activation(ef[:], z9[:], AF.Exp, bias=nmxf[:, 0:1], accum_out=se[:])
            lgz = crf.tile([BL, 1], f32, name="lgz", tag="lgz")
            nc.scalar.activation(lgz[:], se[:], AF.Ln)
            za = crf.tile([BL, 1], f32, name="za", tag="za")
            nc.vector.tensor_tensor(za[:], lgz[:], nmxf[:], op=OP.subtract)  # logZ
            zb = crf.tile([BL, 1], f32, name="zb", tag="zb")
            nc.vector.tensor_tensor(zb[:], za[:], unary[:], op=OP.subtract)
            nll = crf.tile([BL, 1], f32, name="nll", tag="nll")
            nc.vector.tensor_tensor(nll[:], zb[:], binry[:], op=OP.subtract)
            pf = pp.tile([1, 1], f32, name="pf", tag="psmall")
            nc.tensor.matmul(pf[:], nll[:], onesb[:], start=True, stop=True)
            osb = crf.tile([1, 1], f32, name="osb", tag="osb")
            nc.scalar.copy(osb[:], pf[:])
            nc.sync.dma_start(out_d[:, :], osb[:])

        PH = cfg.get("phase", 99)

        def probe(src_ap):
            pt = cp.tile([1, 1], f32, name="probe", tag="probe")
            nc.sync.dma_start(pt[:], src_ap)
            nc.sync.dma_start(out_d[:, :], pt[:])

        for _rep in range(cfg.get("repeat", 1)):
            lstm_layer(0, embT, NKE, x1t)
            if PH == 1:
                probe(x1t[0:1, 0:1].bitcast(f32))
            if PH >= 2:
                lstm_layer(1, x1t, NKX, x2t)
                if PH == 2:
                    probe(x2t[0:1, 0:1].bitcast(f32))
            if PH >= 3:
                logits_and_crf()

    if split:
        _split_excess_waits(nc)
    return nc




# ---------------------------------------------------------------- host prep
def _prep_core(emb_c, lens_c, tgt_c, weights, cfg):
    Tn, BL, En, Hn, Kn = cfg["T"], cfg["BL"], cfg["E"], cfg["H"], cfg["K"]
    EP = -(-En // 128) * 128
    G4 = 4 * Hn
    HP = 2 * Hn
    NKH = Hn // 128
    NKX = HP // 128
    ROWS = Tn * BL
    K2 = Kn * Kn

    # gate column order [f, i, j, o] (TF weight order is [i, j, f, o])
    perm = np.concatenate(
        [np.arange(2 * Hn, 3 * Hn), np.arange(0, Hn),
         np.arange(Hn, 2 * Hn), np.arange(3 * Hn, 4 * Hn)]
    )

    def prep_wb(w, b):
        wp = np.ascontiguousarray(w[:, perm], np.float32)
        bp = b[perm].astype(np.float32).copy()
        bp[0:Hn] += 1.0  # forget_bias on the f block
        return wp, bp

    def chunk_k(w, kpad):
        out = np.zeros((kpad, w.shape[1]), np.float32)
        out[: w.shape[0]] = w
        nk = kpad // 128
        return np.ascontiguousarray(
            out.reshape(nk, 128, w.shape[1]).transpose(1, 0, 2).reshape(128, -1)
        )

    d = {}
    et = emb_c.transpose(2, 1, 0).reshape(En, ROWS)
    embT = np.zeros((EP, ROWS), np.float32)
    embT[:En] = et
    embT[En] = 1.0  # ones-row: carries the layer-0 bias through the pre-GEMM
    d["embT"] = embT

    tt = np.arange(Tn)
    m_fw = (tt[None, :] < lens_c[:, None]).astype(np.float32)
    m_bw = ((Tn - 1 - tt)[None, :] < lens_c[:, None]).astype(np.float32)
    d["m2"] = np.concatenate([m_fw, m_bw], axis=0)

    ohm = np.zeros((ROWS, Kn), np.float32)
    r = tt[:, None] * BL + np.arange(BL)[None, :]          # [T, BL] row ids
    ohm[r.ravel(), tgt_c.T.ravel()] = (tt[:, None] < lens_c[None, :]).astype(
        np.float32
    ).ravel()
    d["oh"] = ohm

    c81 = np.zeros((K2, BL), np.float32)
    for b in range(BL):
        L = int(lens_c[b])
        for t in range(L - 1):
            c81[tgt_c[b, t] * Kn + tgt_c[b, t + 1], b] += 1.0
    d["c81t"] = c81

    d["sel"] = (np.arange(128)[:, None] % BL == np.arange(BL)[None, :]).astype(np.float32)
    d["identt"] = np.tile(np.eye(BL, dtype=np.float32), (128 // BL, 1))
    B2 = 2 * BL
    TPC = 128 // BL
    p = np.arange(128)
    idf = np.zeros((128, TPC * B2), np.float32)
    idb = np.zeros((128, TPC * B2), np.float32)
    for j in range(TPC):
        rows = np.arange(BL * j, BL * (j + 1))
        idf[rows, j * B2 + np.arange(BL)] = 1.0
        idb[rows, j * B2 + BL + np.arange(BL)] = 1.0
    d["idf"] = idf
    d["idb"] = idb
    trans = weights["trans"]
    d["tr81"] = np.concatenate(
        [np.tile(trans.T.reshape(1, K2), (BL, 1)),
         np.tile(trans.reshape(1, K2), (BL, 1))], axis=0
    ).astype(np.float32)
    Mh = Tn // 2
    rr = np.arange(Mh + 1)
    m_alpha = (rr[None, :] < lens_c[:, None])
    m_beta = ((Tn - rr)[None, :] < lens_c[:, None]) & (rr[None, :] <= Mh - 1)
    d["mcrf"] = np.concatenate(
        [m_alpha.astype(np.float32), m_beta.astype(np.float32)], axis=0
    )
    d["transflat"] = trans.reshape(K2, 1).astype(np.float32)
    dwp = chunk_k(weights["dense_w"].astype(np.float32), HP)
    d["dwc"] = dwp
    d["db"] = weights["dense_b"].reshape(1, Kn).astype(np.float32)
    d["ones1"] = np.ones((1, 128), np.float32)

    for l, (wfk, bfk, wbk, bbk, kin) in enumerate(
        (("w_fw0", "b_fw0", "w_bw0", "b_bw0", EP), ("w_fw1", "b_fw1", "w_bw1", "b_bw1", HP))
    ):
        wh_parts = []
        for dd, (wk, bk) in (("f", (wfk, bfk)), ("b", (wbk, bbk))):
            w, b = prep_wb(weights[wk], weights[bk])
            wx_part = w[: w.shape[0] - Hn]      # input rows
            wh_parts.append(w[w.shape[0] - Hn :])  # recurrent rows (last H)
            if l == 0:
                # bias rides the embT ones-row (row En of the padded chunk)
                wx_part = np.concatenate([wx_part, b.reshape(1, G4)], axis=0)
            d[f"wx{l}{dd}"] = chunk_k(wx_part, kin)
            d[f"bias{l}{dd}"] = b.reshape(1, G4)
        d[f"wh{l}"] = np.concatenate(
            [chunk_k(p, Hn) for p in wh_parts], axis=1
        )
    return d


def _get_runner(cfg):
    key = ("runner", cfg["T"], cfg["BL"], cfg["n_cores"], cfg.get("repeat", 1))
    if key in _CACHE:
        return _CACHE[key]
    nc = build_nc(cfg)
    from concourse import bass2jax

    n_cores = cfg["n_cores"]

    import jax
    import numpy as _np
    from jax.sharding import Mesh, PartitionSpec
    from jax.experimental.shard_map import shard_map

    bass2jax.install_neuronx_cc_hook()
    partition_name = nc.partition_id_tensor.name if nc.partition_id_tensor else None
    import concourse.mybir as mybir

    in_names, out_names, out_avals, zero_shapes = [], [], [], []
    for alloc in nc.m.functions[0].allocations:
        if not isinstance(alloc, mybir.MemoryLocationSet):
            continue
        name = alloc.memorylocations[0].name
        if alloc.kind == "ExternalInput":
            if name != partition_name:
                in_names.append(name)
        elif alloc.kind == "ExternalOutput":
            out_names.append(name)
            out_avals.append(
                jax.core.ShapedArray(tuple(alloc.tensor_shape), mybir.dt.np(alloc.dtype))
            )
    n_params = len(in_names)
    all_names = in_names + out_names
    if partition_name is not None:
        all_names = all_names + [partition_name]
    donate = tuple(range(n_params, n_params + len(out_names)))

    def _body(*args):
        operands = list(args)
        if partition_name is not None:
            operands.append(bass2jax.partition_id_tensor())
        outs = bass2jax._bass_exec_p.bind(
            *operands,
            out_avals=tuple(out_avals),
            in_names=tuple(all_names),
            out_names=tuple(out_names),
            lowering_input_output_aliases=(),
            sim_require_finite=True,
            sim_require_nnan=True,
            nc=nc,
        )
        return tuple(outs)

    devices = jax.devices()[:n_cores]

    class Runner:
        pass

    r = Runner()
    r.in_names, r.out_names, r.out_avals, r.n_cores = in_names, out_names, out_avals, n_cores
    if n_cores == 1:
        fn = jax.jit(_body, donate_argnums=donate, keep_unused=True)

        def pack(in_maps):
            return [np.asarray(in_maps[0][n]) for n in in_names]

        def call(packed):
            zeros = [np.zeros(a.shape, a.dtype) for a in out_avals]
            outs = fn(*packed, *zeros)
            return [{n: np.asarray(outs[i]) for i, n in enumerate(out_names)}]
    else:
        from jax.sharding import NamedSharding

        mesh = Mesh(_np.asarray(devices), ("core",))
        fn = jax.jit(
            shard_map(
                _body,
                mesh=mesh,
                in_specs=(PartitionSpec("core"),) * (n_params + len(out_names)),
                out_specs=(PartitionSpec("core"),) * len(out_names),
                check_rep=False,
            ),
            donate_argnums=donate,
            keep_unused=True,
        )
        sh = NamedSharding(mesh, PartitionSpec("core"))

        def pack(in_maps):
            concat_in = [
                np.concatenate([np.asarray(m[n]) for m in in_maps], axis=0)
                for n in in_names
            ]
            return [jax.device_put(a, sh) for a in concat_in]

        def call(packed):
            zeros = [
                np.zeros((n_cores * a.shape[0],) + tuple(a.shape[1:]), a.dtype)
                for a in out_avals
            ]
            outs = fn(*packed, *zeros)
            return [
                {
                    n: np.asarray(outs[i]).reshape((n_cores,) + tuple(out_avals[i].shape))[c]
                    for i, n in enumerate(out_names)
                }
                for c in range(n_cores)
            ]

    r.fn = fn
    r.pack = pack
    r.call = call

    def run(in_maps):
        return call(pack(in_maps))

    r.run = run
    _CACHE[key] = r
    return r


def make_in_maps(inputs, cfg):
    n_cores = cfg["n_cores"]
    BL = cfg["BL"]
    weights = {
        k: np.asarray(inputs[k], np.float32)
        for k in (
            "w_fw0", "b_fw0", "w_bw0", "b_bw0",
            "w_fw1", "b_fw1", "w_bw1", "b_bw1",
            "dense_w", "dense_b", "trans",
        )
    }
    emb = np.asarray(inputs["emb"], np.float32)
    lens = np.asarray(inputs["seq_lens"], np.int64)
    tgt = np.asarray(inputs["targets"], np.int64)
    in_maps = []
    for c in range(n_cores):
        sl = slice(c * BL, (c + 1) * BL)
        in_maps.append(_prep_core(emb[sl], lens[sl], tgt[sl], weights, cfg))
    return in_maps


def kernel(**inputs):
    cfg = dict(T=T, BL=B // N_CORES, E=E, H=H, K=K, n_cores=N_CORES)
    in_maps = make_in_maps(inputs, cfg)
    runner = _get_runner(cfg)
    res = runner.run(in_maps)
    total = sum(float(r["out"][0, 0]) for r in res)
    return np.asarray(np.float32(total / B))


# revision 46
# speedup vs baseline: 1.4456x; 1.4456x over previous
"""Bass/Trainium2 kernel for nn_BiCRFModel: 2-layer BiLSTM + dense + CRF NLL.

Strategy (8-core pure data parallelism, 32 sequences/core):
  - Gate-input projections (x @ Wx + b) computed as per-row-chunk pre-GEMMs
    whose emission is INTERLEAVED with the LSTM step loop (chunk ci, then
    steps 4ci..4ci+3), so the PE fills its idle slots with pre-GEMM work,
    stays continuously busy (full p-state clock), and no separate pre-GEMM
    wall-time exists.  Pre-GEMM results go PSUM -> DRAM directly.
  - LSTM recurrence in "orientation A": batch(+both directions) in the
    partition dim (64 rows), gates in the free dim, gate column order
    [f, i, j, o].  Recurrent matmuls use hT as the stationary operand,
    f32r dtype, accumulating into two 512-wide PSUM banks (bank0 = f,i;
    bank1 = j,o) so activations can start after half the matmuls.
  - The per-step gate input xw is injected into PSUM via an identity
    matmul (f32r, off the h critical path) instead of a DVE add.
  - Backward direction = global time flip + per-step state masking
    (c,h *= [t < len]), which reproduces tf.reverse_sequence semantics
    exactly without any per-sequence gather.
  - Per-step PE transposes maintain hT and build the transposed layer
    output X{1,2}T in HBM for the next layer / dense layer.
  - CRF forward recurrence via a [32, 81] logsumexp (alpha_i + trans_ij),
    per-step validity masking; gold path scores via host-built one-hot /
    pair-count tensors contracted on device against logits / trans.
Output: per-core sum of NLL over its 32 sequences; host sums and /256.
"""

import contextlib

import numpy as np

B, T, E, H, K = 256, 256, 300, 256, 9
N_CORES = 8

_CACHE = {}


# ---------------------------------------------------------------- wait split
def _split_excess_waits(nc, max_waits=1):
    """This walrus build allows only 1 sync wait per instruction.  Hoist
    excess waits onto InstEventSemaphore carriers inserted just before the
    instruction (same engine -> same program order -> identical blocking)."""
    import bass_rust
    import concourse.mybir as mybir

    n_split = 0
    for fn in nc.m.functions:
        for bb in fn.blocks:
            insts = list(bb.instructions)
            out = []
            changed = False
            for ins in insts:
                si = getattr(ins, "sync_info", None)
                waits = list(si.on_wait) if si is not None and si.on_wait else []
                if len(waits) > max_waits:
                    keep = waits[:max_waits]
                    rest = waits[max_waits:]
                    for ci in range(0, len(rest), max_waits):
                        nop = mybir.InstEventSemaphore(
                            name=f"{ins.name}-waitsplit-{ci}", ins=[], outs=[]
                        )
                        nop.engine = ins.engine
                        nop.bass_nofuse = True
                        nop.sync_info = bass_rust.SyncInfo(
                            on_wait=list(rest[ci : ci + max_waits]), on_update=[]
                        )
                        out.append(nop)
                    si.on_wait = keep
                    n_split += 1
                    changed = True
                out.append(ins)
            if changed:
                bb.instructions[:] = out
    return n_split


# ---------------------------------------------------------------- builder
def build_nc(cfg, split=True):
    import concourse.bass as bass
    import concourse.mybir as mybir
    from concourse import tile

    f32 = mybir.dt.float32
    f32r = mybir.dt.float32r
    AF = mybir.ActivationFunctionType
    OP = mybir.AluOpType
    AX = mybir.AxisListType

    Tn = cfg["T"]
    BL = cfg["BL"]
    En = cfg["E"]
    Hn = cfg["H"]
    Kn = cfg["K"]
    EP = -(-En // 128) * 128          # padded input feat
    G4 = 4 * Hn                        # gate width
    HP = 2 * Hn                        # concat feat
    B2 = 2 * BL                        # fw+bw packed batch
    NKE = EP // 128
    NKH = Hn // 128
    NKX = HP // 128
    ROWS = Tn * BL
    NCH = ROWS // 128                  # row chunks
    TPC = 128 // BL                    # timesteps per chunk
    NB = G4 // 512                     # psum n-slices
    K2 = Kn * Kn
    NKH2 = 2 * NKH

    nc = bass.Bass("TRN2", num_devices=cfg["n_cores"])

    embT = nc.dram_tensor("embT", [EP, ROWS], f32r, kind="ExternalInput")
    m2_d = nc.dram_tensor("m2", [B2, Tn], f32, kind="ExternalInput")
    oh_d = nc.dram_tensor("oh", [ROWS, Kn], f32, kind="ExternalInput")
    c81_d = nc.dram_tensor("c81t", [K2, BL], f32, kind="ExternalInput")
    sel_d = nc.dram_tensor("sel", [128, BL], f32, kind="ExternalInput")
    id_d = nc.dram_tensor("identt", [128, BL], f32, kind="ExternalInput")
    idf_d = nc.dram_tensor("idf", [128, TPC * B2], f32r, kind="ExternalInput")
    idb_d = nc.dram_tensor("idb", [128, TPC * B2], f32r, kind="ExternalInput")
    tr81_d = nc.dram_tensor("tr81", [B2, K2], f32, kind="ExternalInput")
    mcrf_d = nc.dram_tensor("mcrf", [B2, Tn // 2 + 1], f32, kind="ExternalInput")
    tf_d = nc.dram_tensor("transflat", [K2, 1], f32, kind="ExternalInput")
    dw_d = nc.dram_tensor("dwc", [128, NKX * Kn], f32r, kind="ExternalInput")
    db_d = nc.dram_tensor("db", [1, Kn], f32r, kind="ExternalInput")
    on_d = nc.dram_tensor("ones1", [1, 128], f32r, kind="ExternalInput")
    wx_d, wh_d, bias_d = {}, {}, {}
    for l in (0, 1):
        nk = NKE if l == 0 else NKX
        for d in ("f", "b"):
            wx_d[(l, d)] = nc.dram_tensor(f"wx{l}{d}", [128, nk * G4], f32r, kind="ExternalInput")
            bias_d[(l, d)] = nc.dram_tensor(f"bias{l}{d}", [1, G4], f32r, kind="ExternalInput")
        wh_d[l] = nc.dram_tensor(f"wh{l}", [128, NKH2 * G4], f32r, kind="ExternalInput")
    out_d = nc.dram_tensor("out", [1, 1], f32, kind="ExternalOutput")

    with tile.TileContext(nc) as tc, contextlib.ExitStack() as ctx:
        cp = ctx.enter_context(tc.tile_pool(name="const", bufs=1))
        gp = ctx.enter_context(tc.tile_pool(name="work", bufs=2))
        sp = ctx.enter_context(tc.tile_pool(name="step", bufs=2))
        pp = ctx.enter_context(tc.tile_pool(name="psum", bufs=2, space="PSUM"))
        dp = ctx.enter_context(tc.tile_pool(name="dram", bufs=1, space="DRAM"))
        crf = ctx.enter_context(tc.tile_pool(name="crf", bufs=2))

        def cload(name, dram, shape, dt=f32):
            t = cp.tile(shape, dt, name=name, tag=name)
            nc.sync.dma_start(t[:], dram[:, :])
            return t

        m2s = cload("m2s", m2_d, [B2, Tn])
        c81s = cload("c81s", c81_d, [K2, BL])
        sels = cload("sels", sel_d, [128, BL])
        ids = cload("ids", id_d, [128, BL])
        idfs = cload("idfs", idf_d, [128, TPC * B2], f32r)
        idbs = cload("idbs", idb_d, [128, TPC * B2], f32r)
        tr81s = cload("tr81s", tr81_d, [B2, K2])
        mcrfs = cload("mcrfs", mcrf_d, [B2, Tn // 2 + 1])
        tfs = cload("tfs", tf_d, [K2, 1])
        wxs, whs, bss = {}, {}, {}
        for l in (0, 1):
            nk = NKE if l == 0 else NKX
            for d in ("f", "b"):
                wxs[(l, d)] = cload(f"wxs{l}{d}", wx_d[(l, d)], [128, nk * G4], f32r)
                bss[(l, d)] = cload(f"bss{l}{d}", bias_d[(l, d)], [1, G4], f32r)
            whs[l] = cload(f"whs{l}", wh_d[l], [128, NKH2 * G4], f32r)
        ones1 = cload("ones1s", on_d, [1, 128], f32r)
        onesb = cp.tile([BL, 1], f32, name="onesb", tag="onesb")
        nc.vector.memset(onesb[:], 1.0)

        x1t = dp.tile([HP, ROWS], f32r, name="x1t", tag="x1t")
        x2t = dp.tile([HP, ROWS], f32r, name="x2t", tag="x2t")
        lgd = dp.tile([ROWS, Kn], f32, name="lgd", tag="lgd")

        # ------------- pre-GEMM chunk: zs{f,b} = srcT.T @ Wx + b, kept in SBUF
        # zs rows are the 128 (4 timesteps x 32 batch) rows of the chunk; the
        # per-step identity matmuls read 32-row slices directly.
        def pre_gemm_chunk(l, src, nk, ci):
            out = {}
            for d in ("f", "b"):
                c = ci if d == "f" else NCH - 1 - ci
                xt = gp.tile([128, nk * 128], f32r, name=f"pgx{l}{d}{ci}", tag="pgx", bufs=3)
                nc.sync.dma_start(
                    xt[:].rearrange("p (k m) -> p k m", k=nk),
                    src[:, 128 * c : 128 * (c + 1)].rearrange("(k p) m -> p k m", k=nk),
                )
                zs = gp.tile([128, G4], f32r, name=f"pgs{l}{d}{ci}", tag=f"zs{d}", bufs=3)
                pbanks = (
                    (pp.tile([128, 512], f32, name=f"pgz{l}{d}{ci}_0", tag="zp0"), 0, 512),
                    (pp.tile([128, Hn], f32, name=f"pgzj{l}{d}{ci}", tag="zpj", bufs=1), 512, Hn),
                    (pp.tile([128, Hn], f32, name=f"pgzo{l}{d}{ci}", tag="zpo", bufs=1), 768, Hn),
                )
                for zpb, n0, nw in pbanks:
                    # layer 0 rides the bias on embT's ones-row (row E); layer 1
                    # needs an explicit rank-1 bias matmul.
                    if l != 0:
                        nc.tensor.matmul(
                            zpb[:], ones1[:], bss[(l, d)][:, n0 : n0 + nw],
                            start=True, stop=False,
                        )
                    for k in range(nk):
                        nc.tensor.matmul(
                            zpb[:],
                            xt[:, 128 * k : 128 * (k + 1)],
                            wxs[(l, d)][:, k * G4 + n0 : k * G4 + n0 + nw],
                            start=(l == 0 and k == 0),
                            stop=(k == nk - 1),
                        )
                for idx, (zpb, n0, nw) in enumerate(pbanks):
                    if idx == 0:
                        nc.scalar.copy(zs[:, n0 : n0 + nw], zpb[:])
                    else:
                        nc.vector.tensor_copy(zs[:, n0 : n0 + nw], zpb[:])
                out[d] = zs
            return out

        # ------------- one LSTM step (both dirs packed in 64 partitions)
        # Recurrent matmul uses a block-diagonal stationary operand so both
        # directions land in one M=64 base-0 PSUM write (f32r cannot write
        # PSUM at a partition offset): lhsT k-chunks 0..NKH-1 hold hT_fw in
        # cols 0:BL (rest zero), chunks NKH..2NKH-1 hold hT_bw in cols BL:2BL.
        def hT_dst(hTbig, di):
            return hTbig[:, di * NKH * B2 : (di + 1) * NKH * B2].rearrange(
                "p (c x) -> p c x", c=NKH
            )[:, :, di * BL : (di + 1) * BL]

        def lstm_step(l, s, c_prev, hTbig, xout, zsd):
            tfw, tbw = s, Tn - 1 - s
            j = s % TPC            # row band of zs['f'] for this step
            jb = TPC - 1 - j       # row band of zs['b'] (time-reversed chunk)

            # PSUM: bank0 = [f, i] (512), bankJ = [j] (256), bankO = [o] (256).
            # PE order: bank0's matmuls first (sigmoid f,i is the longest ACT
            # op), then bankJ (tanh j gates the c-chain), then bankO.
            zp0 = pp.tile([B2, 512], f32, name=f"slz{l}_{s}_0", tag="zp0")
            zpj = pp.tile([B2, Hn], f32, name=f"slzj{l}_{s}", tag="zpj", bufs=1)
            zpo = pp.tile([B2, Hn], f32, name=f"slzo{l}_{s}", tag="zpo", bufs=1)
            banks = ((zp0, 0, 512), (zpj, 512, 256), (zpo, 768, 256))
            for zpb, n0, nw in banks:
                nc.tensor.matmul(
                    zpb[:], idfs[:, B2 * j : B2 * (j + 1)],
                    zsd["f"][:, n0 : n0 + nw],
                    start=True, stop=False,
                )
                nc.tensor.matmul(
                    zpb[:], idbs[:, B2 * jb : B2 * (jb + 1)],
                    zsd["b"][:, n0 : n0 + nw],
                    start=False, stop=False,
                )
            for zpb, n0, nw in banks:
                for k in range(NKH2):
                    nc.tensor.matmul(
                        zpb[:],
                        hTbig[:, B2 * k : B2 * (k + 1)],
                        whs[l][:, k * G4 + n0 : k * G4 + n0 + nw],
                        start=False,
                        stop=(k == NKH2 - 1),
                    )

            # gate order [f, i | j | o]
            sfi = sp.tile([B2, 2 * Hn], f32, name=f"sfi{l}_{s}", tag="sfi")
            nc.scalar.activation(sfi[:], zp0[:], AF.Sigmoid)
            g = sp.tile([B2, Hn], f32, name=f"g{l}_{s}", tag="g")
            nc.scalar.activation(g[:], zpj[:], AF.Tanh)
            so = sp.tile([B2, Hn], f32, name=f"so{l}_{s}", tag="so")
            nc.scalar.activation(so[:], zpo[:], AF.Sigmoid)

            mcol = m2s[:, s : s + 1]
            t2 = sp.tile([B2, Hn], f32, name=f"t2{l}_{s}", tag="t2")
            nc.vector.scalar_tensor_tensor(
                t2[:], sfi[:, 0:Hn], mcol, c_prev[:], OP.mult, OP.mult
            )
            # the tail runs as two feature-half chains so tanh/h/transpose
            # pipeline between ACT, DVE and PE
            HH = Hn // 2
            c_new = sp.tile([B2, Hn], f32, name=f"c{l}_{s}", tag="cst", bufs=3)
            t1 = sp.tile([B2, Hn], f32, name=f"t1{l}_{s}", tag="t1")
            th = sp.tile([B2, Hn], f32, name=f"th{l}_{s}", tag="th")
            h = sp.tile([B2, Hn], f32, name=f"h{l}_{s}", tag="h")
            tps = []
            for di in range(2):
                tp = pp.tile(
                    [128, NKH * BL], f32, name=f"tp{di}_{l}_{s}",
                    tag=f"tp{di}", bufs=1,
                )
                tps.append(tp)
            for q in range(2):
                ql = slice(HH * q, HH * (q + 1))
                nc.vector.scalar_tensor_tensor(
                    t1[:, ql], sfi[:, Hn + HH * q : Hn + HH * (q + 1)], mcol,
                    g[:, ql], OP.mult, OP.mult,
                )
                nc.vector.tensor_tensor(c_new[:, ql], t1[:, ql], t2[:, ql], op=OP.add)
                nc.scalar.activation(th[:, ql], c_new[:, ql], AF.Tanh)
                nc.vector.scalar_tensor_tensor(
                    h[:, ql], so[:, ql], mcol, th[:, ql], OP.mult, OP.mult
                )
                # feature half q == hT k-chunk q: transpose as soon as ready
                k = q
                for di in range(2):
                    po = BL * di
                    nc.tensor.matmul(
                        tps[di][:, 32 * k : 32 * k + 32],
                        h[po : po + BL, 128 * k : 128 * (k + 1)],
                        ids[po : po + BL, 0:BL],
                        is_transpose=True,
                    )
                # copy each k-chunk into hTbig immediately; the next step's
                # k-chunk matmuls unblock per chunk (subtile deps)
                nc.scalar.copy(
                    hT_dst(hTbig, 0)[:, k : k + 1, :],
                    tps[0][:, 32 * k : 32 * k + 32].rearrange(
                        "p (c x) -> p c x", c=1
                    ),
                )
                nc.vector.tensor_copy(
                    hT_dst(hTbig, 1)[:, k : k + 1, :],
                    tps[1][:, 32 * k : 32 * k + 32].rearrange(
                        "p (c x) -> p c x", c=1
                    ),
                )
            for di, tdst in ((0, tfw), (1, tbw)):
                nc.gpsimd.dma_start(
                    xout[
                        Hn * di : Hn * (di + 1), BL * tdst : BL * (tdst + 1)
                    ].rearrange("(k p) b -> p k b", k=NKH),
                    hT_dst(hTbig, di),
                )
            return c_new

        # ------------- a full BiLSTM layer: pre-GEMM interleaved with steps
        def lstm_layer(l, src, nk, xout, post_chunk=None):
            z0 = sp.tile([128, NKH2 * B2], f32, name=f"z0_{l}", tag="z0")
            nc.vector.memset(z0[:], 0.0)
            hTbig = sp.tile([128, NKH2 * B2], f32r, name=f"hTbig{l}", tag="hTbig", bufs=1)
            nc.scalar.copy(hTbig[:], z0[:])
            c_prev = sp.tile([B2, Hn], f32, name=f"cinit{l}", tag="cst", bufs=3)
            nc.vector.memset(c_prev[:], 0.0)
            nsteps = min(Tn, cfg.get("nsteps", Tn))
            for ci in range(NCH):
                zsd = pre_gemm_chunk(l, src, nk, ci)
                for s in range(TPC * ci, min(TPC * (ci + 1), nsteps)):
                    c_prev = lstm_step(l, s, c_prev, hTbig, xout, zsd)
                if post_chunk is not None:
                    post_chunk(ci)
                if TPC * (ci + 1) >= nsteps:
                    break

        LST = {}

        def logits_setup():
            dws32 = cp.tile([128, NKX * Kn], f32, name="dws32", tag="dws32")
            nc.sync.dma_start(dws32[:], dw_d[:, :].bitcast(f32))
            dbs32 = cp.tile([1, Kn], f32, name="dbs32", tag="dbs32")
            nc.sync.dma_start(dbs32[:], db_d[:, :].bitcast(f32))
            on32 = cp.tile([1, 128], f32, name="on32", tag="on32")
            nc.sync.dma_start(on32[:], on_d[:, :].bitcast(f32))
            usum = cp.tile([128, NCH], f32, name="usum", tag="usum")
            ohall = cp.tile([128, NCH * Kn], f32, name="ohall", tag="ohall")
            nc.sync.dma_start(
                ohall[:].rearrange("p (c k) -> p c k", c=NCH),
                oh_d[:, :].rearrange("(c p) k -> p c k", c=NCH),
            )
            LST.update(dws32=dws32, dbs32=dbs32, on32=on32, usum=usum,
                       ohall=ohall, done=set())

        def logits_chunk(c):
            # logits for x2t column-chunk c (times 4c..4c+3); emitted inline
            # into layer 1's step loop once those columns are stored
            LST["done"].add(c)
            lp = pp.tile([128, Kn], f32, name=f"lp{c}", tag="psmall")
            nc.tensor.matmul(lp[:], LST["on32"][:], LST["dbs32"][:], start=True, stop=False)
            xt = gp.tile([128, NKX * 128], f32, name=f"lgx{c}", tag="lgx", bufs=4)
            nc.sync.dma_start(
                xt[:].rearrange("p (k m) -> p k m", k=NKX),
                x2t[:, 128 * c : 128 * (c + 1)].bitcast(f32).rearrange(
                    "(k p) m -> p k m", k=NKX
                ),
            )
            for k in range(NKX):
                nc.tensor.matmul(
                    lp[:],
                    xt[:, 128 * k : 128 * (k + 1)],
                    LST["dws32"][:, Kn * k : Kn * (k + 1)],
                    start=False,
                    stop=(k == NKX - 1),
                )
            lgc = cp.tile([128, Kn], f32, name=f"lg{c}", tag=f"lg{c}")
            nc.vector.tensor_copy(lgc[:], lp[:])
            nc.sync.dma_start(lgd[128 * c : 128 * (c + 1), :], lgc[:])
            scr = gp.tile([128, Kn], f32, name=f"ohscr{c}", tag="ohscr")
            nc.vector.scalar_tensor_tensor(
                scr[:], lgc[:], 1.0, LST["ohall"][:, Kn * c : Kn * (c + 1)],
                OP.mult, OP.mult,
                accum_out=LST["usum"][:, c : c + 1],
            )

        def l1_post_chunk(ci):
            if ci >= NCH // 2:
                for c in (NCH - 1 - ci, ci):
                    if c not in LST["done"]:
                        logits_chunk(c)

        def logits_and_crf():
            for c in range(NCH):
                if c not in LST["done"]:
                    logits_chunk(c)
            usum = LST["usum"]

            # ---------------- gold-path scores
            up = pp.tile([BL, NCH], f32, name="up", tag="psmall")
            nc.tensor.matmul(up[:], sels[:], usum[:], start=True, stop=True)
            unary = cp.tile([BL, 1], f32, name="unary", tag="unary")
            nc.vector.reduce_sum(unary[:], up[:], axis=AX.X)
            bp = pp.tile([BL, 1], f32, name="bp", tag="psmall")
            nc.tensor.matmul(bp[:], c81s[:], tfs[:], start=True, stop=True)
            binry = cp.tile([BL, 1], f32, name="binry", tag="binry")
            nc.scalar.copy(binry[:], bp[:])

            # ---------------- CRF forward/backward split recurrence
            # Rows 0:BL run the forward alpha over t=0..M; rows BL:2BL run the
            # backward beta over t=T-1..M (M = T/2).  128 packed iterations,
            # then logZ = lse(alpha_M + beta_M).  Instead of a per-iteration
            # max-subtraction, exp uses a running -mx bias renormalized every
            # RENORM iterations (f32 exp headroom covers the drift).
            M = Tn // 2
            RENORM = 8
            # lgpost: rows 0:BL = logits blocks t=0..M (alpha post-add);
            #         rows BL:  = 0
            # lgpre:  rows BL:2BL = logits blocks t=0..255 (beta pre-add,
            #         indexed at 256-r); rows 0:BL = 0
            lgpost = cp.tile([B2, (M + 1) * Kn], f32, name="lgpost", tag="lgpost")
            nc.vector.memset(lgpost[:], 0.0)
            nc.sync.dma_start(
                lgpost[0:BL, :].rearrange("b (t k) -> b t k", k=Kn),
                lgd[0 : (M + 1) * BL, :].rearrange("(t b) k -> b t k", b=BL),
            )
            lgpre = cp.tile([B2, Tn * Kn], f32, name="lgpre", tag="lgpre")
            nc.vector.memset(lgpre[0:BL, :], 0.0)
            nc.sync.dma_start(
                lgpre[BL:B2, :].rearrange("b (t k) -> b t k", k=Kn),
                lgd[:, :].rearrange("(t b) k -> b t k", b=BL),
            )
            S = crf.tile([B2, Kn], f32, name="S0", tag="S")
            nc.vector.memset(S[BL:B2, :], 0.0)
            nc.vector.tensor_copy(S[0:BL, :], lgpost[0:BL, 0:Kn])
            mx = None
            for r in range(1, M + 1):
                if (r - 1) % RENORM == 0:
                    mx = crf.tile([B2, 1], f32, name=f"mx{r}", tag="mx")
                    nc.vector.reduce_max(mx[:], S[:], axis=AX.X)
                    nmx = crf.tile([B2, 1], f32, name=f"nmx{r}", tag="nmx")
                    nc.vector.tensor_scalar_mul(nmx[:], mx[:], -1.0)
                pre = crf.tile([B2, Kn], f32, name=f"pre{r}", tag="pre")
                nc.vector.tensor_tensor(
                    pre[:], S[:], lgpre[:, Kn * (Tn - r) : Kn * (Tn - r + 1)],
                    op=OP.add,
                )
                a81 = crf.tile([B2, K2], f32, name=f"a81_{r}", tag="a81")
                nc.vector.tensor_tensor(
                    a81[:].rearrange("p (j i) -> p j i", i=Kn),
                    pre[:].unsqueeze(1).broadcast_to([B2, Kn, Kn]),
                    tr81s[:].rearrange("p (j i) -> p j i", i=Kn),
                    op=OP.add,
                )
                e81 = crf.tile([B2, K2], f32, name=f"e81_{r}", tag="e81")
                nc.scalar.activation(e81[:], a81[:], AF.Exp, bias=nmx[:, 0:1])
                s9 = crf.tile([B2, Kn], f32, name=f"s9_{r}", tag="s9")
                nc.vector.reduce_sum(
                    s9[:], e81[:].rearrange("p (j i) -> p j i", i=Kn), axis=AX.X
                )
                lgs = crf.tile([B2, Kn], f32, name=f"lgs{r}", tag="lgs")
                nc.scalar.activation(lgs[:], s9[:], AF.Ln)
                cand = crf.tile([B2, Kn], f32, name=f"cand{r}", tag="cand")
                nc.vector.scalar_tensor_tensor(
                    cand[:], lgs[:], nmx[:, 0:1], lgpost[:, Kn * r : Kn * (r + 1)],
                    OP.subtract, OP.add,
                )
                dd = crf.tile([B2, Kn], f32, name=f"dd{r}", tag="dd")
                nc.vector.tensor_tensor(dd[:], cand[:], S[:], op=OP.subtract)
                snew = crf.tile([B2, Kn], f32, name=f"S{r}", tag="S")
                nc.vector.scalar_tensor_tensor(
                    snew[:], dd[:], mcrfs[:, r : r + 1], S[:], OP.mult, OP.add
                )
                S = snew

            # ---------------- logZ = lse(alpha_M + beta_M), nll, partial sum
            bet = crf.tile([BL, Kn], f32, name="bet", tag="bet")
            nc.vector.tensor_copy(bet[:], S[BL:B2, :])
            z9 = crf.tile([BL, Kn], f32, name="z9", tag="z9")
            nc.vector.tensor_tensor(z9[:], S[0:BL, :], bet[:], op=OP.add)
            mxf = crf.tile([BL, 1], f32, name="mxf", tag="mxf")
            nc.vector.reduce_max(mxf[:], z9[:], axis=AX.X)
            nmxf = crf.tile([BL, 1], f32, name="nmxf", tag="nmxf")
            nc.vector.tensor_scalar_mul(nmxf[:], mxf[:], -1.0)
            ef = crf.tile([BL, Kn], f32, name="ef", tag="ef")
            se = crf.tile([BL, 1], f32, name="se", tag="se")
            nc.scalar.activation(ef[:], z9[:], AF.Exp, bias=nmxf[:, 0:1], accum_out=se[:])
            lgz = crf.tile([BL, 1], f32, name="lgz", tag="lgz")
            nc.scalar.activation(lgz[:], se[:], AF.Ln)
            za = crf.tile([BL, 1], f32, name="za", tag="za")
            nc.vector.tensor_tensor(za[:], lgz[:], nmxf[:], op=OP.subtract)  # logZ
            zb = crf.tile([BL, 1], f32, name="zb", tag="zb")
            nc.vector.tensor_tensor(zb[:], za[:], unary[:], op=OP.subtract)
            nll = crf.tile([BL, 1], f32, name="nll", tag="nll")
            nc.vector.tensor_tensor(nll[:], zb[:], binry[:], op=OP.subtract)
            pf = pp.tile([1, 1], f32, name="pf", tag="psmall")
            nc.tensor.matmul(pf[:], nll[:], onesb[:], start=True, stop=True)
            osb = crf.tile([1, 1], f32, name="osb", tag="osb")
            nc.scalar.copy(osb[:], pf[:])
            nc.sync.dma_start(out_d[:, :], osb[:])

        PH = cfg.get("phase", 99)

        def probe(src_ap):
            pt = cp.tile([1, 1], f32, name="probe", tag="probe")
            nc.sync.dma_start(pt[:], src_ap)
            nc.sync.dma_start(out_d[:, :], pt[:])

        for _rep in range(cfg.get("repeat", 1)):
            lstm_layer(0, embT, NKE, x1t)
            if PH == 1:
                probe(x1t[0:1, 0:1].bitcast(f32))
            if PH >= 2:
                if PH >= 3:
                    logits_setup()
                    lstm_layer(1, x1t, NKX, x2t, post_chunk=l1_post_chunk)
                    logits_and_crf()
                else:
                    lstm_layer(1, x1t, NKX, x2t)
                    probe(x2t[0:1, 0:1].bitcast(f32))

    if split:
        _split_excess_waits(nc)
    return nc




# ---------------------------------------------------------------- host prep
def _prep_core(emb_c, lens_c, tgt_c, weights, cfg):
    Tn, BL, En, Hn, Kn = cfg["T"], cfg["BL"], cfg["E"], cfg["H"], cfg["K"]
    EP = -(-En // 128) * 128
    G4 = 4 * Hn
    HP = 2 * Hn
    NKH = Hn // 128
    NKX = HP // 128
    ROWS = Tn * BL
    K2 = Kn * Kn

    # gate column order [f, i, j, o] (TF weight order is [i, j, f, o])
    perm = np.concatenate(
        [np.arange(2 * Hn, 3 * Hn), np.arange(0, Hn),
         np.arange(Hn, 2 * Hn), np.arange(3 * Hn, 4 * Hn)]
    )

    def prep_wb(w, b):
        wp = np.ascontiguousarray(w[:, perm], np.float32)
        bp = b[perm].astype(np.float32).copy()
        bp[0:Hn] += 1.0  # forget_bias on the f block
        return wp, bp

    def chunk_k(w, kpad):
        out = np.zeros((kpad, w.shape[1]), np.float32)
        out[: w.shape[0]] = w
        nk = kpad // 128
        return np.ascontiguousarray(
            out.reshape(nk, 128, w.shape[1]).transpose(1, 0, 2).reshape(128, -1)
        )

    d = {}
    et = emb_c.transpose(2, 1, 0).reshape(En, ROWS)
    embT = np.zeros((EP, ROWS), np.float32)
    embT[:En] = et
    embT[En] = 1.0  # ones-row: carries the layer-0 bias through the pre-GEMM
    d["embT"] = embT

    tt = np.arange(Tn)
    m_fw = (tt[None, :] < lens_c[:, None]).astype(np.float32)
    m_bw = ((Tn - 1 - tt)[None, :] < lens_c[:, None]).astype(np.float32)
    d["m2"] = np.concatenate([m_fw, m_bw], axis=0)

    ohm = np.zeros((ROWS, Kn), np.float32)
    r = tt[:, None] * BL + np.arange(BL)[None, :]          # [T, BL] row ids
    ohm[r.ravel(), tgt_c.T.ravel()] = (tt[:, None] < lens_c[None, :]).astype(
        np.float32
    ).ravel()
    d["oh"] = ohm

    c81 = np.zeros((K2, BL), np.float32)
    for b in range(BL):
        L = int(lens_c[b])
        for t in range(L - 1):
            c81[tgt_c[b, t] * Kn + tgt_c[b, t + 1], b] += 1.0
    d["c81t"] = c81

    d["sel"] = (np.arange(128)[:, None] % BL == np.arange(BL)[None, :]).astype(np.float32)
    d["identt"] = np.tile(np.eye(BL, dtype=np.float32), (128 // BL, 1))
    B2 = 2 * BL
    TPC = 128 // BL
    p = np.arange(128)
    idf = np.zeros((128, TPC * B2), np.float32)
    idb = np.zeros((128, TPC * B2), np.float32)
    for j in range(TPC):
        rows = np.arange(BL * j, BL * (j + 1))
        idf[rows, j * B2 + np.arange(BL)] = 1.0
        idb[rows, j * B2 + BL + np.arange(BL)] = 1.0
    d["idf"] = idf
    d["idb"] = idb
    trans = weights["trans"]
    d["tr81"] = np.concatenate(
        [np.tile(trans.T.reshape(1, K2), (BL, 1)),
         np.tile(trans.reshape(1, K2), (BL, 1))], axis=0
    ).astype(np.float32)
    Mh = Tn // 2
    rr = np.arange(Mh + 1)
    m_alpha = (rr[None, :] < lens_c[:, None])
    m_beta = ((Tn - rr)[None, :] < lens_c[:, None]) & (rr[None, :] <= Mh - 1)
    d["mcrf"] = np.concatenate(
        [m_alpha.astype(np.float32), m_beta.astype(np.float32)], axis=0
    )
    d["transflat"] = trans.reshape(K2, 1).astype(np.float32)
    dwp = chunk_k(weights["dense_w"].astype(np.float32), HP)
    d["dwc"] = dwp
    d["db"] = weights["dense_b"].reshape(1, Kn).astype(np.float32)
    d["ones1"] = np.ones((1, 128), np.float32)

    for l, (wfk, bfk, wbk, bbk, kin) in enumerate(
        (("w_fw0", "b_fw0", "w_bw0", "b_bw0", EP), ("w_fw1", "b_fw1", "w_bw1", "b_bw1", HP))
    ):
        wh_parts = []
        for dd, (wk, bk) in (("f", (wfk, bfk)), ("b", (wbk, bbk))):
            w, b = prep_wb(weights[wk], weights[bk])
            wx_part = w[: w.shape[0] - Hn]      # input rows
            wh_parts.append(w[w.shape[0] - Hn :])  # recurrent rows (last H)
            if l == 0:
                # bias rides the embT ones-row (row En of the padded chunk)
                wx_part = np.concatenate([wx_part, b.reshape(1, G4)], axis=0)
            d[f"wx{l}{dd}"] = chunk_k(wx_part, kin)
            d[f"bias{l}{dd}"] = b.reshape(1, G4)
        d[f"wh{l}"] = np.concatenate(
            [chunk_k(p, Hn) for p in wh_parts], axis=1
        )
    return d


def _get_runner(cfg):
    key = ("runner", cfg["T"], cfg["BL"], cfg["n_cores"], cfg.get("repeat", 1))
    if key in _CACHE:
        return _CACHE[key]
    nc = build_nc(cfg)
    from concourse import bass2jax

    n_cores = cfg["n_cores"]

    import jax
    import numpy as _np
    from jax.sharding import Mesh, PartitionSpec
    from jax.experimental.shard_map import shard_map

    bass2jax.install_neuronx_cc_hook()
    partition_name = nc.partition_id_tensor.name if nc.partition_id_tensor else None
    import concourse.mybir as mybir

    in_names, out_names, out_avals, zero_shapes = [], [], [], []
    for alloc in nc.m.functions[0].allocations:
        if not isinstance(alloc, mybir.MemoryLocationSet):
            continue
        name = alloc.memorylocations[0].name
        if alloc.kind == "ExternalInput":
            if name != partition_name:
                in_names.append(name)
        elif alloc.kind == "ExternalOutput":
            out_names.append(name)
            out_avals.append(
                jax.core.ShapedArray(tuple(alloc.tensor_shape), mybir.dt.np(alloc.dtype))
            )
    n_params = len(in_names)
    all_names = in_names + out_names
    if partition_name is not None:
        all_names = all_names + [partition_name]
    donate = tuple(range(n_params, n_params + len(out_names)))

    def _body(*args):
        operands = list(args)
        if partition_name is not None:
            operands.append(bass2jax.partition_id_tensor())
        outs = bass2jax._bass_exec_p.bind(
            *operands,
            out_avals=tuple(out_avals),
            in_names=tuple(all_names),
            out_names=tuple(out_names),
            lowering_input_output_aliases=(),
            sim_require_finite=True,
            sim_require_nnan=True,
            nc=nc,
        )
        return tuple(outs)

    devices = jax.devices()[:n_cores]

    class Runner:
        pass

    r = Runner()
    r.in_names, r.out_names, r.out_avals, r.n_cores = in_names, out_names, out_avals, n_cores
    if n_cores == 1:
        fn = jax.jit(_body, donate_argnums=donate, keep_unused=True)

        def pack(in_maps):
            return [np.asarray(in_maps[0][n]) for n in in_names]

        def call(packed):
            zeros = [np.zeros(a.shape, a.dtype) for a in out_avals]
            outs = fn(*packed, *zeros)
            return [{n: np.asarray(outs[i]) for i, n in enumerate(out_names)}]
    else:
        from jax.sharding import NamedSharding

        mesh = Mesh(_np.asarray(devices), ("core",))
        fn = jax.jit(
            shard_map(
                _body,
                mesh=mesh,
                in_specs=(PartitionSpec("core"),) * (n_params + len(out_names)),
                out_specs=(PartitionSpec("core"),) * len(out_names),
                check_rep=False,
            ),
            donate_argnums=donate,
            keep_unused=True,
        )
        sh = NamedSharding(mesh, PartitionSpec("core"))

        def pack(in_maps):
            concat_in = [
                np.concatenate([np.asarray(m[n]) for m in in_maps], axis=0)
                for n in in_names
            ]
            return [jax.device_put(a, sh) for a in concat_in]

        def call(packed):
            zeros = [
                np.zeros((n_cores * a.shape[0],) + tuple(a.shape[1:]), a.dtype)
                for a in out_avals
            ]
            outs = fn(*packed, *zeros)
            return [
                {
                    n: np.asarray(outs[i]).reshape((n_cores,) + tuple(out_avals[i].shape))[c]
                    for i, n in enumerate(out_names)
                }
                for c in range(n_cores)
            ]

    r.fn = fn
    r.pack = pack
    r.call = call

    def run(in_maps):
        return call(pack(in_maps))

    r.run = run
    _CACHE[key] = r
    return r


def make_in_maps(inputs, cfg):
    n_cores = cfg["n_cores"]
    BL = cfg["BL"]
    weights = {
        k: np.asarray(inputs[k], np.float32)
        for k in (
            "w_fw0", "b_fw0", "w_bw0", "b_bw0",
            "w_fw1", "b_fw1", "w_bw1", "b_bw1",
            "dense_w", "dense_b", "trans",
        )
    }
    emb = np.asarray(inputs["emb"], np.float32)
    lens = np.asarray(inputs["seq_lens"], np.int64)
    tgt = np.asarray(inputs["targets"], np.int64)
    in_maps = []
    for c in range(n_cores):
        sl = slice(c * BL, (c + 1) * BL)
        in_maps.append(_prep_core(emb[sl], lens[sl], tgt[sl], weights, cfg))
    return in_maps


def kernel(**inputs):
    cfg = dict(T=T, BL=B // N_CORES, E=E, H=H, K=K, n_cores=N_CORES)
    in_maps = make_in_maps(inputs, cfg)
    runner = _get_runner(cfg)
    res = runner.run(in_maps)
    total = sum(float(r["out"][0, 0]) for r in res)
    return np.asarray(np.float32(total / B))


# revision 47
# speedup vs baseline: 1.6392x; 1.1339x over previous
"""Bass/Trainium2 kernel for nn_BiCRFModel: 2-layer BiLSTM + dense + CRF NLL.

Strategy (8-core pure data parallelism, 32 sequences/core):
  - Gate-input projections (x @ Wx + b) computed as per-row-chunk pre-GEMMs
    whose emission is INTERLEAVED with the LSTM step loop (chunk ci, then
    steps 4ci..4ci+3), so the PE fills its idle slots with pre-GEMM work,
    stays continuously busy (full p-state clock), and no separate pre-GEMM
    wall-time exists.  Pre-GEMM results go PSUM -> DRAM directly.
  - LSTM recurrence in "orientation A": batch(+both directions) in the
    partition dim (64 rows), gates in the free dim, gate column order
    [f, i, j, o].  Recurrent matmuls use hT as the stationary operand,
    f32r dtype, accumulating into two 512-wide PSUM banks (bank0 = f,i;
    bank1 = j,o) so activations can start after half the matmuls.
  - The per-step gate input xw is injected into PSUM via an identity
    matmul (f32r, off the h critical path) instead of a DVE add.
  - Backward direction = global time flip + per-step state masking
    (c,h *= [t < len]), which reproduces tf.reverse_sequence semantics
    exactly without any per-sequence gather.
  - Per-step PE transposes maintain hT and build the transposed layer
    output X{1,2}T in HBM for the next layer / dense layer.
  - CRF forward recurrence via a [32, 81] logsumexp (alpha_i + trans_ij),
    per-step validity masking; gold path scores via host-built one-hot /
    pair-count tensors contracted on device against logits / trans.
Output: per-core sum of NLL over its 32 sequences; host sums and /256.
"""

import contextlib

import numpy as np

B, T, E, H, K = 256, 256, 300, 256, 9
N_CORES = 8

_CACHE = {}


# ---------------------------------------------------------------- wait split
def _split_excess_waits(nc, max_waits=1):
    """This walrus build allows only 1 sync wait per instruction.  Hoist
    excess waits onto InstEventSemaphore carriers inserted just before the
    instruction (same engine -> same program order -> identical blocking)."""
    import bass_rust
    import concourse.mybir as mybir

    n_split = 0
    for fn in nc.m.functions:
        for bb in fn.blocks:
            insts = list(bb.instructions)
            out = []
            changed = False
            for ins in insts:
                si = getattr(ins, "sync_info", None)
                waits = list(si.on_wait) if si is not None and si.on_wait else []
                if len(waits) > max_waits:
                    keep = waits[:max_waits]
                    rest = waits[max_waits:]
                    for ci in range(0, len(rest), max_waits):
                        nop = mybir.InstEventSemaphore(
                            name=f"{ins.name}-waitsplit-{ci}", ins=[], outs=[]
                        )
                        nop.engine = ins.engine
                        nop.bass_nofuse = True
                        nop.sync_info = bass_rust.SyncInfo(
                            on_wait=list(rest[ci : ci + max_waits]), on_update=[]
                        )
                        out.append(nop)
                    si.on_wait = keep
                    n_split += 1
                    changed = True
                out.append(ins)
            if changed:
                bb.instructions[:] = out
    return n_split


# ---------------------------------------------------------------- builder
def build_nc(cfg, split=True):
    import concourse.bass as bass
    import concourse.mybir as mybir
    from concourse import tile

    f32 = mybir.dt.float32
    f32r = mybir.dt.float32r
    AF = mybir.ActivationFunctionType
    OP = mybir.AluOpType
    AX = mybir.AxisListType

    Tn = cfg["T"]
    BL = cfg["BL"]
    En = cfg["E"]
    Hn = cfg["H"]
    Kn = cfg["K"]
    EP = -(-En // 128) * 128          # padded input feat
    G4 = 4 * Hn                        # gate width
    HP = 2 * Hn                        # concat feat
    B2 = 2 * BL                        # fw+bw packed batch
    NKE = EP // 128
    NKH = Hn // 128
    NKX = HP // 128
    ROWS = Tn * BL
    NCH = ROWS // 128                  # row chunks
    TPC = 128 // BL                    # timesteps per chunk
    NB = G4 // 512                     # psum n-slices
    K2 = Kn * Kn
    NKH2 = 2 * NKH

    nc = bass.Bass("TRN2", num_devices=cfg["n_cores"])

    embT = nc.dram_tensor("embT", [EP, ROWS], f32r, kind="ExternalInput")
    m2_d = nc.dram_tensor("m2", [B2, Tn], f32, kind="ExternalInput")
    oh_d = nc.dram_tensor("oh", [ROWS, Kn], f32, kind="ExternalInput")
    c81_d = nc.dram_tensor("c81t", [K2, BL], f32, kind="ExternalInput")
    sel_d = nc.dram_tensor("sel", [128, BL], f32, kind="ExternalInput")
    id_d = nc.dram_tensor("identt", [128, BL], f32, kind="ExternalInput")
    idf_d = nc.dram_tensor("idf", [128, TPC * B2], f32r, kind="ExternalInput")
    idb_d = nc.dram_tensor("idb", [128, TPC * B2], f32r, kind="ExternalInput")
    tr81_d = nc.dram_tensor("tr81", [B2, K2], f32, kind="ExternalInput")
    mcrf_d = nc.dram_tensor("mcrf", [B2, Tn // 2 + 1], f32, kind="ExternalInput")
    tf_d = nc.dram_tensor("transflat", [K2, 1], f32, kind="ExternalInput")
    dw_d = nc.dram_tensor("dwc", [128, NKX * Kn], f32r, kind="ExternalInput")
    db_d = nc.dram_tensor("db", [1, Kn], f32r, kind="ExternalInput")
    on_d = nc.dram_tensor("ones1", [1, 128], f32r, kind="ExternalInput")
    wx_d, wh_d, bias_d = {}, {}, {}
    for l in (0, 1):
        nk = NKE if l == 0 else NKX
        for d in ("f", "b"):
            wx_d[(l, d)] = nc.dram_tensor(f"wx{l}{d}", [128, nk * G4], f32r, kind="ExternalInput")
            bias_d[(l, d)] = nc.dram_tensor(f"bias{l}{d}", [1, G4], f32r, kind="ExternalInput")
        wh_d[l] = nc.dram_tensor(f"wh{l}", [128, NKH2 * G4], f32r, kind="ExternalInput")
    out_d = nc.dram_tensor("out", [1, 1], f32, kind="ExternalOutput")

    with tile.TileContext(nc) as tc, contextlib.ExitStack() as ctx:
        cp = ctx.enter_context(tc.tile_pool(name="const", bufs=1))
        gp = ctx.enter_context(tc.tile_pool(name="work", bufs=2))
        sp = ctx.enter_context(tc.tile_pool(name="step", bufs=2))
        pp = ctx.enter_context(tc.tile_pool(name="psum", bufs=2, space="PSUM"))
        dp = ctx.enter_context(tc.tile_pool(name="dram", bufs=1, space="DRAM"))
        crf = ctx.enter_context(tc.tile_pool(name="crf", bufs=2))

        def cload(name, dram, shape, dt=f32):
            t = cp.tile(shape, dt, name=name, tag=name)
            nc.sync.dma_start(t[:], dram[:, :])
            return t

        m2s = cload("m2s", m2_d, [B2, Tn])
        c81s = cload("c81s", c81_d, [K2, BL])
        sels = cload("sels", sel_d, [128, BL])
        ids = cload("ids", id_d, [128, BL])
        idfs = cload("idfs", idf_d, [128, TPC * B2], f32r)
        idbs = cload("idbs", idb_d, [128, TPC * B2], f32r)
        tr81s = cload("tr81s", tr81_d, [B2, K2])
        mcrfs = cload("mcrfs", mcrf_d, [B2, Tn // 2 + 1])
        tfs = cload("tfs", tf_d, [K2, 1])
        wxs, whs, bss = {}, {}, {}
        for l in (0, 1):
            nk = NKE if l == 0 else NKX
            for d in ("f", "b"):
                wxs[(l, d)] = cload(f"wxs{l}{d}", wx_d[(l, d)], [128, nk * G4], f32r)
                bss[(l, d)] = cload(f"bss{l}{d}", bias_d[(l, d)], [1, G4], f32r)
            whs[l] = cload(f"whs{l}", wh_d[l], [128, NKH2 * G4], f32r)
        ones1 = cload("ones1s", on_d, [1, 128], f32r)
        onesb = cp.tile([BL, 1], f32, name="onesb", tag="onesb")
        nc.vector.memset(onesb[:], 1.0)

        x1t = dp.tile([HP, ROWS], f32r, name="x1t", tag="x1t")
        x2t = dp.tile([HP, ROWS], f32r, name="x2t", tag="x2t")
        lgd = dp.tile([ROWS, Kn], f32, name="lgd", tag="lgd")

        # ------------- pre-GEMM chunk: zs{f,b} = srcT.T @ Wx + b, kept in SBUF
        # zs rows are the 128 (4 timesteps x 32 batch) rows of the chunk; the
        # per-step identity matmuls read 32-row slices directly.
        def pre_gemm_chunk(l, src, nk, ci):
            out = {}
            for d in ("f", "b"):
                c = ci if d == "f" else NCH - 1 - ci
                xt = gp.tile([128, nk * 128], f32r, name=f"pgx{l}{d}{ci}", tag="pgx", bufs=3)
                nc.sync.dma_start(
                    xt[:].rearrange("p (k m) -> p k m", k=nk),
                    src[:, 128 * c : 128 * (c + 1)].rearrange("(k p) m -> p k m", k=nk),
                )
                zs = gp.tile([128, G4], f32r, name=f"pgs{l}{d}{ci}", tag=f"zs{d}", bufs=3)
                pbanks = (
                    (pp.tile([128, 512], f32, name=f"pgz{l}{d}{ci}_0", tag="zp0"), 0, 512),
                    (pp.tile([128, Hn], f32, name=f"pgzj{l}{d}{ci}", tag="zpj", bufs=1), 512, Hn),
                    (pp.tile([128, Hn], f32, name=f"pgzo{l}{d}{ci}", tag="zpo", bufs=1), 768, Hn),
                )
                for zpb, n0, nw in pbanks:
                    # layer 0 rides the bias on embT's ones-row (row E); layer 1
                    # needs an explicit rank-1 bias matmul.
                    if l != 0:
                        nc.tensor.matmul(
                            zpb[:], ones1[:], bss[(l, d)][:, n0 : n0 + nw],
                            start=True, stop=False,
                        )
                    for k in range(nk):
                        nc.tensor.matmul(
                            zpb[:],
                            xt[:, 128 * k : 128 * (k + 1)],
                            wxs[(l, d)][:, k * G4 + n0 : k * G4 + n0 + nw],
                            start=(l == 0 and k == 0),
                            stop=(k == nk - 1),
                        )
                for idx, (zpb, n0, nw) in enumerate(pbanks):
                    if idx == 0:
                        nc.scalar.copy(zs[:, n0 : n0 + nw], zpb[:])
                    else:
                        nc.vector.tensor_copy(zs[:, n0 : n0 + nw], zpb[:])
                out[d] = zs
            return out

        # ------------- one LSTM step (both dirs packed in 64 partitions)
        # Recurrent matmul uses a block-diagonal stationary operand so both
        # directions land in one M=64 base-0 PSUM write (f32r cannot write
        # PSUM at a partition offset): lhsT k-chunks 0..NKH-1 hold hT_fw in
        # cols 0:BL (rest zero), chunks NKH..2NKH-1 hold hT_bw in cols BL:2BL.
        def hT_dst(hTbig, di):
            return hTbig[:, di * NKH * B2 : (di + 1) * NKH * B2].rearrange(
                "p (c x) -> p c x", c=NKH
            )[:, :, di * BL : (di + 1) * BL]

        def lstm_step(l, s, c_prev, hTbig, xout, zsd):
            tfw, tbw = s, Tn - 1 - s
            j = s % TPC            # row band of zs['f'] for this step
            jb = TPC - 1 - j       # row band of zs['b'] (time-reversed chunk)

            # PSUM: bank0 = [f, i] (512), bankJ = [j] (256), bankO = [o] (256).
            # PE order: bank0's matmuls first (sigmoid f,i is the longest ACT
            # op), then bankJ (tanh j gates the c-chain), then bankO.
            zp0 = pp.tile([B2, 512], f32, name=f"slz{l}_{s}_0", tag="zp0")
            zpj = pp.tile([B2, Hn], f32, name=f"slzj{l}_{s}", tag="zpj", bufs=1)
            zpo = pp.tile([B2, Hn], f32, name=f"slzo{l}_{s}", tag="zpo", bufs=1)
            banks = ((zp0, 0, 512), (zpj, 512, 256), (zpo, 768, 256))
            # complete each bank's accumulation group before starting the next
            # so sigmoid(f,i) / tanh(j) unblock as early as possible
            for zpb, n0, nw in banks:
                nc.tensor.matmul(
                    zpb[:], idfs[:, B2 * j : B2 * (j + 1)],
                    zsd["f"][:, n0 : n0 + nw],
                    start=True, stop=False,
                )
                nc.tensor.matmul(
                    zpb[:], idbs[:, B2 * jb : B2 * (jb + 1)],
                    zsd["b"][:, n0 : n0 + nw],
                    start=False, stop=False,
                )
                for k in range(NKH2):
                    nc.tensor.matmul(
                        zpb[:],
                        hTbig[:, B2 * k : B2 * (k + 1)],
                        whs[l][:, k * G4 + n0 : k * G4 + n0 + nw],
                        start=False,
                        stop=(k == NKH2 - 1),
                    )

            # gate order [f, i | j | o]
            sfi = sp.tile([B2, 2 * Hn], f32, name=f"sfi{l}_{s}", tag="sfi")
            nc.scalar.activation(sfi[:], zp0[:], AF.Sigmoid)
            g = sp.tile([B2, Hn], f32, name=f"g{l}_{s}", tag="g")
            nc.scalar.activation(g[:], zpj[:], AF.Tanh)
            so = sp.tile([B2, Hn], f32, name=f"so{l}_{s}", tag="so")
            nc.scalar.activation(so[:], zpo[:], AF.Sigmoid)

            mcol = m2s[:, s : s + 1]
            t2 = sp.tile([B2, Hn], f32, name=f"t2{l}_{s}", tag="t2")
            nc.vector.scalar_tensor_tensor(
                t2[:], sfi[:, 0:Hn], mcol, c_prev[:], OP.mult, OP.mult
            )
            # the tail runs as two feature-half chains so tanh/h/transpose
            # pipeline between ACT, DVE and PE
            HH = Hn // 2
            c_new = sp.tile([B2, Hn], f32, name=f"c{l}_{s}", tag="cst", bufs=3)
            t1 = sp.tile([B2, Hn], f32, name=f"t1{l}_{s}", tag="t1")
            th = sp.tile([B2, Hn], f32, name=f"th{l}_{s}", tag="th")
            h = sp.tile([B2, Hn], f32, name=f"h{l}_{s}", tag="h")
            tps = []
            for di in range(2):
                tp = pp.tile(
                    [128, NKH * BL], f32, name=f"tp{di}_{l}_{s}",
                    tag=f"tp{di}", bufs=1,
                )
                tps.append(tp)
            for q in range(2):
                ql = slice(HH * q, HH * (q + 1))
                nc.vector.scalar_tensor_tensor(
                    t1[:, ql], sfi[:, Hn + HH * q : Hn + HH * (q + 1)], mcol,
                    g[:, ql], OP.mult, OP.mult,
                )
                nc.vector.tensor_tensor(c_new[:, ql], t1[:, ql], t2[:, ql], op=OP.add)
                nc.scalar.activation(th[:, ql], c_new[:, ql], AF.Tanh)
                nc.vector.scalar_tensor_tensor(
                    h[:, ql], so[:, ql], mcol, th[:, ql], OP.mult, OP.mult
                )
                # feature half q == hT k-chunk q: transpose as soon as ready
                k = q
                for di in range(2):
                    po = BL * di
                    nc.tensor.matmul(
                        tps[di][:, 32 * k : 32 * k + 32],
                        h[po : po + BL, 128 * k : 128 * (k + 1)],
                        ids[po : po + BL, 0:BL],
                        is_transpose=True,
                    )
                # copy each k-chunk into hTbig immediately; the next step's
                # k-chunk matmuls unblock per chunk (subtile deps)
                nc.scalar.copy(
                    hT_dst(hTbig, 0)[:, k : k + 1, :],
                    tps[0][:, 32 * k : 32 * k + 32].rearrange(
                        "p (c x) -> p c x", c=1
                    ),
                )
                nc.vector.tensor_copy(
                    hT_dst(hTbig, 1)[:, k : k + 1, :],
                    tps[1][:, 32 * k : 32 * k + 32].rearrange(
                        "p (c x) -> p c x", c=1
                    ),
                )
            for di, tdst in ((0, tfw), (1, tbw)):
                nc.gpsimd.dma_start(
                    xout[
                        Hn * di : Hn * (di + 1), BL * tdst : BL * (tdst + 1)
                    ].rearrange("(k p) b -> p k b", k=NKH),
                    hT_dst(hTbig, di),
                )
            return c_new

        # ------------- a full BiLSTM layer: pre-GEMM interleaved with steps
        def lstm_layer(l, src, nk, xout, post_chunk=None):
            z0 = sp.tile([128, NKH2 * B2], f32, name=f"z0_{l}", tag="z0")
            nc.vector.memset(z0[:], 0.0)
            hTbig = sp.tile([128, NKH2 * B2], f32r, name=f"hTbig{l}", tag="hTbig", bufs=1)
            nc.scalar.copy(hTbig[:], z0[:])
            c_prev = sp.tile([B2, Hn], f32, name=f"cinit{l}", tag="cst", bufs=3)
            nc.vector.memset(c_prev[:], 0.0)
            nsteps = min(Tn, cfg.get("nsteps", Tn))
            for ci in range(NCH):
                zsd = pre_gemm_chunk(l, src, nk, ci)
                for s in range(TPC * ci, min(TPC * (ci + 1), nsteps)):
                    c_prev = lstm_step(l, s, c_prev, hTbig, xout, zsd)
                if post_chunk is not None:
                    post_chunk(ci)
                if TPC * (ci + 1) >= nsteps:
                    break

        LST = {}

        def logits_setup():
            dws32 = cp.tile([128, NKX * Kn], f32, name="dws32", tag="dws32")
            nc.sync.dma_start(dws32[:], dw_d[:, :].bitcast(f32))
            dbs32 = cp.tile([1, Kn], f32, name="dbs32", tag="dbs32")
            nc.sync.dma_start(dbs32[:], db_d[:, :].bitcast(f32))
            on32 = cp.tile([1, 128], f32, name="on32", tag="on32")
            nc.sync.dma_start(on32[:], on_d[:, :].bitcast(f32))
            usum = cp.tile([128, NCH], f32, name="usum", tag="usum")
            ohall = cp.tile([128, NCH * Kn], f32, name="ohall", tag="ohall")
            nc.sync.dma_start(
                ohall[:].rearrange("p (c k) -> p c k", c=NCH),
                oh_d[:, :].rearrange("(c p) k -> p c k", c=NCH),
            )
            LST.update(dws32=dws32, dbs32=dbs32, on32=on32, usum=usum,
                       ohall=ohall, done=set())

        def logits_chunk(c):
            # logits for x2t column-chunk c (times 4c..4c+3); emitted inline
            # into layer 1's step loop once those columns are stored
            LST["done"].add(c)
            lp = pp.tile([128, Kn], f32, name=f"lp{c}", tag="psmall")
            nc.tensor.matmul(lp[:], LST["on32"][:], LST["dbs32"][:], start=True, stop=False)
            xt = gp.tile([128, NKX * 128], f32, name=f"lgx{c}", tag="lgx", bufs=4)
            nc.sync.dma_start(
                xt[:].rearrange("p (k m) -> p k m", k=NKX),
                x2t[:, 128 * c : 128 * (c + 1)].bitcast(f32).rearrange(
                    "(k p) m -> p k m", k=NKX
                ),
            )
            for k in range(NKX):
                nc.tensor.matmul(
                    lp[:],
                    xt[:, 128 * k : 128 * (k + 1)],
                    LST["dws32"][:, Kn * k : Kn * (k + 1)],
                    start=False,
                    stop=(k == NKX - 1),
                )
            lgc = cp.tile([128, Kn], f32, name=f"lg{c}", tag=f"lg{c}")
            nc.vector.tensor_copy(lgc[:], lp[:])
            nc.sync.dma_start(lgd[128 * c : 128 * (c + 1), :], lgc[:])
            scr = gp.tile([128, Kn], f32, name=f"ohscr{c}", tag="ohscr")
            nc.vector.scalar_tensor_tensor(
                scr[:], lgc[:], 1.0, LST["ohall"][:, Kn * c : Kn * (c + 1)],
                OP.mult, OP.mult,
                accum_out=LST["usum"][:, c : c + 1],
            )

        def l1_post_chunk(ci):
            if ci >= NCH // 2:
                for c in (NCH - 1 - ci, ci):
                    if c not in LST["done"]:
                        logits_chunk(c)

        def logits_and_crf():
            for c in range(NCH):
                if c not in LST["done"]:
                    logits_chunk(c)
            usum = LST["usum"]

            # ---------------- gold-path scores
            up = pp.tile([BL, NCH], f32, name="up", tag="psmall")
            nc.tensor.matmul(up[:], sels[:], usum[:], start=True, stop=True)
            unary = cp.tile([BL, 1], f32, name="unary", tag="unary")
            nc.vector.reduce_sum(unary[:], up[:], axis=AX.X)
            bp = pp.tile([BL, 1], f32, name="bp", tag="psmall")
            nc.tensor.matmul(bp[:], c81s[:], tfs[:], start=True, stop=True)
            binry = cp.tile([BL, 1], f32, name="binry", tag="binry")
            nc.scalar.copy(binry[:], bp[:])

            # ---------------- CRF forward/backward split recurrence
            # Rows 0:BL run the forward alpha over t=0..M; rows BL:2BL run the
            # backward beta over t=T-1..M (M = T/2).  128 packed iterations,
            # then logZ = lse(alpha_M + beta_M).  Instead of a per-iteration
            # max-subtraction, exp uses a running -mx bias renormalized every
            # RENORM iterations (f32 exp headroom covers the drift).
            M = Tn // 2
            RENORM = 8
            # lgpost: rows 0:BL = logits blocks t=0..M (alpha post-add);
            #         rows BL:  = 0
            # lgpre:  rows BL:2BL = logits blocks t=0..255 (beta pre-add,
            #         indexed at 256-r); rows 0:BL = 0
            lgpost = cp.tile([B2, (M + 1) * Kn], f32, name="lgpost", tag="lgpost")
            nc.vector.memset(lgpost[:], 0.0)
            nc.sync.dma_start(
                lgpost[0:BL, :].rearrange("b (t k) -> b t k", k=Kn),
                lgd[0 : (M + 1) * BL, :].rearrange("(t b) k -> b t k", b=BL),
            )
            lgpre = cp.tile([B2, Tn * Kn], f32, name="lgpre", tag="lgpre")
            nc.vector.memset(lgpre[0:BL, :], 0.0)
            nc.sync.dma_start(
                lgpre[BL:B2, :].rearrange("b (t k) -> b t k", k=Kn),
                lgd[:, :].rearrange("(t b) k -> b t k", b=BL),
            )
            S = crf.tile([B2, Kn], f32, name="S0", tag="S")
            nc.vector.memset(S[BL:B2, :], 0.0)
            nc.vector.tensor_copy(S[0:BL, :], lgpost[0:BL, 0:Kn])
            mx = None
            for r in range(1, M + 1):
                if (r - 1) % RENORM == 0:
                    mx = crf.tile([B2, 1], f32, name=f"mx{r}", tag="mx")
                    nc.vector.reduce_max(mx[:], S[:], axis=AX.X)
                    nmx = crf.tile([B2, 1], f32, name=f"nmx{r}", tag="nmx")
                    nc.vector.tensor_scalar_mul(nmx[:], mx[:], -1.0)
                pre = crf.tile([B2, Kn], f32, name=f"pre{r}", tag="pre")
                nc.vector.tensor_tensor(
                    pre[:], S[:], lgpre[:, Kn * (Tn - r) : Kn * (Tn - r + 1)],
                    op=OP.add,
                )
                a81 = crf.tile([B2, K2], f32, name=f"a81_{r}", tag="a81")
                nc.vector.tensor_tensor(
                    a81[:].rearrange("p (j i) -> p j i", i=Kn),
                    pre[:].unsqueeze(1).broadcast_to([B2, Kn, Kn]),
                    tr81s[:].rearrange("p (j i) -> p j i", i=Kn),
                    op=OP.add,
                )
                e81 = crf.tile([B2, K2], f32, name=f"e81_{r}", tag="e81")
                nc.scalar.activation(e81[:], a81[:], AF.Exp, bias=nmx[:, 0:1])
                s9 = crf.tile([B2, Kn], f32, name=f"s9_{r}", tag="s9")
                nc.vector.reduce_sum(
                    s9[:], e81[:].rearrange("p (j i) -> p j i", i=Kn), axis=AX.X
                )
                lgs = crf.tile([B2, Kn], f32, name=f"lgs{r}", tag="lgs")
                nc.scalar.activation(lgs[:], s9[:], AF.Ln)
                cand = crf.tile([B2, Kn], f32, name=f"cand{r}", tag="cand")
                nc.vector.scalar_tensor_tensor(
                    cand[:], lgs[:], nmx[:, 0:1], lgpost[:, Kn * r : Kn * (r + 1)],
                    OP.subtract, OP.add,
                )
                dd = crf.tile([B2, Kn], f32, name=f"dd{r}", tag="dd")
                nc.vector.tensor_tensor(dd[:], cand[:], S[:], op=OP.subtract)
                snew = crf.tile([B2, Kn], f32, name=f"S{r}", tag="S")
                nc.vector.scalar_tensor_tensor(
                    snew[:], dd[:], mcrfs[:, r : r + 1], S[:], OP.mult, OP.add
                )
                S = snew

            # ---------------- logZ = lse(alpha_M + beta_M), nll, partial sum
            bet = crf.tile([BL, Kn], f32, name="bet", tag="bet")
            nc.vector.tensor_copy(bet[:], S[BL:B2, :])
            z9 = crf.tile([BL, Kn], f32, name="z9", tag="z9")
            nc.vector.tensor_tensor(z9[:], S[0:BL, :], bet[:], op=OP.add)
            mxf = crf.tile([BL, 1], f32, name="mxf", tag="mxf")
            nc.vector.reduce_max(mxf[:], z9[:], axis=AX.X)
            nmxf = crf.tile([BL, 1], f32, name="nmxf", tag="nmxf")
            nc.vector.tensor_scalar_mul(nmxf[:], mxf[:], -1.0)
            ef = crf.tile([BL, Kn], f32, name="ef", tag="ef")
            se = crf.tile([BL, 1], f32, name="se", tag="se")
            nc.scalar.activation(ef[:], z9[:], AF.Exp, bias=nmxf[:, 0:1], accum_out=se[:])
            lgz = crf.tile([BL, 1], f32, name="lgz", tag="lgz")
            nc.scalar.activation(lgz[:], se[:], AF.Ln)
            za = crf.tile([BL, 1], f32, name="za", tag="za")
            nc.vector.tensor_tensor(za[:], lgz[:], nmxf[:], op=OP.subtract)  # logZ
            zb = crf.tile([BL, 1], f32, name="zb", tag="zb")
            nc.vector.tensor_tensor(zb[:], za[:], unary[:], op=OP.subtract)
            nll = crf.tile([BL, 1], f32, name="nll", tag="nll")
            nc.vector.tensor_tensor(nll[:], zb[:], binry[:], op=OP.subtract)
            pf = pp.tile([1, 1], f32, name="pf", tag="psmall")
            nc.tensor.matmul(pf[:], nll[:], onesb[:], start=True, stop=True)
            osb = crf.tile([1, 1], f32, name="osb", tag="osb")
            nc.scalar.copy(osb[:], pf[:])
            nc.sync.dma_start(out_d[:, :], osb[:])

        PH = cfg.get("phase", 99)

        def probe(src_ap):
            pt = cp.tile([1, 1], f32, name="probe", tag="probe")
            nc.sync.dma_start(pt[:], src_ap)
            nc.sync.dma_start(out_d[:, :], pt[:])

        for _rep in range(cfg.get("repeat", 1)):
            lstm_layer(0, embT, NKE, x1t)
            if PH == 1:
                probe(x1t[0:1, 0:1].bitcast(f32))
            if PH >= 2:
                if PH >= 3:
                    logits_setup()
                    lstm_layer(1, x1t, NKX, x2t, post_chunk=l1_post_chunk)
                    logits_and_crf()
                else:
                    lstm_layer(1, x1t, NKX, x2t)
                    probe(x2t[0:1, 0:1].bitcast(f32))

    if split:
        _split_excess_waits(nc)
    return nc




# ---------------------------------------------------------------- host prep
def _prep_core(emb_c, lens_c, tgt_c, weights, cfg):
    Tn, BL, En, Hn, Kn = cfg["T"], cfg["BL"], cfg["E"], cfg["H"], cfg["K"]
    EP = -(-En // 128) * 128
    G4 = 4 * Hn
    HP = 2 * Hn
    NKH = Hn // 128
    NKX = HP // 128
    ROWS = Tn * BL
    K2 = Kn * Kn

    # gate column order [f, i, j, o] (TF weight order is [i, j, f, o])
    perm = np.concatenate(
        [np.arange(2 * Hn, 3 * Hn), np.arange(0, Hn),
         np.arange(Hn, 2 * Hn), np.arange(3 * Hn, 4 * Hn)]
    )

    def prep_wb(w, b):
        wp = np.ascontiguousarray(w[:, perm], np.float32)
        bp = b[perm].astype(np.float32).copy()
        bp[0:Hn] += 1.0  # forget_bias on the f block
        return wp, bp

    def chunk_k(w, kpad):
        out = np.zeros((kpad, w.shape[1]), np.float32)
        out[: w.shape[0]] = w
        nk = kpad // 128
        return np.ascontiguousarray(
            out.reshape(nk, 128, w.shape[1]).transpose(1, 0, 2).reshape(128, -1)
        )

    d = {}
    et = emb_c.transpose(2, 1, 0).reshape(En, ROWS)
    embT = np.zeros((EP, ROWS), np.float32)
    embT[:En] = et
    embT[En] = 1.0  # ones-row: carries the layer-0 bias through the pre-GEMM
    d["embT"] = embT

    tt = np.arange(Tn)
    m_fw = (tt[None, :] < lens_c[:, None]).astype(np.float32)
    m_bw = ((Tn - 1 - tt)[None, :] < lens_c[:, None]).astype(np.float32)
    d["m2"] = np.concatenate([m_fw, m_bw], axis=0)

    ohm = np.zeros((ROWS, Kn), np.float32)
    r = tt[:, None] * BL + np.arange(BL)[None, :]          # [T, BL] row ids
    ohm[r.ravel(), tgt_c.T.ravel()] = (tt[:, None] < lens_c[None, :]).astype(
        np.float32
    ).ravel()
    d["oh"] = ohm

    c81 = np.zeros((K2, BL), np.float32)
    for b in range(BL):
        L = int(lens_c[b])
        for t in range(L - 1):
            c81[tgt_c[b, t] * Kn + tgt_c[b, t + 1], b] += 1.0
    d["c81t"] = c81

    d["sel"] = (np.arange(128)[:, None] % BL == np.arange(BL)[None, :]).astype(np.float32)
    d["identt"] = np.tile(np.eye(BL, dtype=np.float32), (128 // BL, 1))
    B2 = 2 * BL
    TPC = 128 // BL
    p = np.arange(128)
    idf = np.zeros((128, TPC * B2), np.float32)
    idb = np.zeros((128, TPC * B2), np.float32)
    for j in range(TPC):
        rows = np.arange(BL * j, BL * (j + 1))
        idf[rows, j * B2 + np.arange(BL)] = 1.0
        idb[rows, j * B2 + BL + np.arange(BL)] = 1.0
    d["idf"] = idf
    d["idb"] = idb
    trans = weights["trans"]
    d["tr81"] = np.concatenate(
        [np.tile(trans.T.reshape(1, K2), (BL, 1)),
         np.tile(trans.reshape(1, K2), (BL, 1))], axis=0
    ).astype(np.float32)
    Mh = Tn // 2
    rr = np.arange(Mh + 1)
    m_alpha = (rr[None, :] < lens_c[:, None])
    m_beta = ((Tn - rr)[None, :] < lens_c[:, None]) & (rr[None, :] <= Mh - 1)
    d["mcrf"] = np.concatenate(
        [m_alpha.astype(np.float32), m_beta.astype(np.float32)], axis=0
    )
    d["transflat"] = trans.reshape(K2, 1).astype(np.float32)
    dwp = chunk_k(weights["dense_w"].astype(np.float32), HP)
    d["dwc"] = dwp
    d["db"] = weights["dense_b"].reshape(1, Kn).astype(np.float32)
    d["ones1"] = np.ones((1, 128), np.float32)

    for l, (wfk, bfk, wbk, bbk, kin) in enumerate(
        (("w_fw0", "b_fw0", "w_bw0", "b_bw0", EP), ("w_fw1", "b_fw1", "w_bw1", "b_bw1", HP))
    ):
        wh_parts = []
        for dd, (wk, bk) in (("f", (wfk, bfk)), ("b", (wbk, bbk))):
            w, b = prep_wb(weights[wk], weights[bk])
            wx_part = w[: w.shape[0] - Hn]      # input rows
            wh_parts.append(w[w.shape[0] - Hn :])  # recurrent rows (last H)
            if l == 0:
                # bias rides the embT ones-row (row En of the padded chunk)
                wx_part = np.concatenate([wx_part, b.reshape(1, G4)], axis=0)
            d[f"wx{l}{dd}"] = chunk_k(wx_part, kin)
            d[f"bias{l}{dd}"] = b.reshape(1, G4)
        d[f"wh{l}"] = np.concatenate(
            [chunk_k(p, Hn) for p in wh_parts], axis=1
        )
    return d


def _get_runner(cfg):
    key = ("runner", cfg["T"], cfg["BL"], cfg["n_cores"], cfg.get("repeat", 1))
    if key in _CACHE:
        return _CACHE[key]
    nc = build_nc(cfg)
    from concourse import bass2jax

    n_cores = cfg["n_cores"]

    import jax
    import numpy as _np
    from jax.sharding import Mesh, PartitionSpec
    from jax.experimental.shard_map import shard_map

    bass2jax.install_neuronx_cc_hook()
    partition_name = nc.partition_id_tensor.name if nc.partition_id_tensor else None
    import concourse.mybir as mybir

    in_names, out_names, out_avals, zero_shapes = [], [], [], []
    for alloc in nc.m.functions[0].allocations:
        if not isinstance(alloc, mybir.MemoryLocationSet):
            continue
        name = alloc.memorylocations[0].name
        if alloc.kind == "ExternalInput":
            if name != partition_name:
                in_names.append(name)
        elif alloc.kind == "ExternalOutput":
            out_names.append(name)
            out_avals.append(
                jax.core.ShapedArray(tuple(alloc.tensor_shape), mybir.dt.np(alloc.dtype))
            )
    n_params = len(in_names)
    all_names = in_names + out_names
    if partition_name is not None:
        all_names = all_names + [partition_name]
    donate = tuple(range(n_params, n_params + len(out_names)))

    def _body(*args):
        operands = list(args)
        if partition_name is not None:
            operands.append(bass2jax.partition_id_tensor())
        outs = bass2jax._bass_exec_p.bind(
            *operands,
            out_avals=tuple(out_avals),
            in_names=tuple(all_names),
            out_names=tuple(out_names),
            lowering_input_output_aliases=(),
            sim_require_finite=True,
            sim_require_nnan=True,
            nc=nc,
        )
        return tuple(outs)

    devices = jax.devices()[:n_cores]

    class Runner:
        pass

    r = Runner()
    r.in_names, r.out_names, r.out_avals, r.n_cores = in_names, out_names, out_avals, n_cores
    if n_cores == 1:
        fn = jax.jit(_body, donate_argnums=donate, keep_unused=True)

        def pack(in_maps):
            return [np.asarray(in_maps[0][n]) for n in in_names]

        def call(packed):
            zeros = [np.zeros(a.shape, a.dtype) for a in out_avals]
            outs = fn(*packed, *zeros)
            return [{n: np.asarray(outs[i]) for i, n in enumerate(out_names)}]
    else:
        from jax.sharding import NamedSharding

        mesh = Mesh(_np.asarray(devices), ("core",))
        fn = jax.jit(
            shard_map(
                _body,
                mesh=mesh,
                in_specs=(PartitionSpec("core"),) * (n_params + len(out_names)),
                out_specs=(PartitionSpec("core"),) * len(out_names),
                check_rep=False,
            ),
            donate_argnums=donate,
            keep_unused=True,
        )
        sh = NamedSharding(mesh, PartitionSpec("core"))

        def pack(in_maps):
            concat_in = [
                np.concatenate([np.asarray(m[n]) for m in in_maps], axis=0)
                for n in in_names
            ]
            return [jax.device_put(a, sh) for a in concat_in]

        def call(packed):
            zeros = [
                np.zeros((n_cores * a.shape[0],) + tuple(a.shape[1:]), a.dtype)
                for a in out_avals
            ]
            outs = fn(*packed, *zeros)
            return [
                {
                    n: np.asarray(outs[i]).reshape((n_cores,) + tuple(out_avals[i].shape))[c]
                    for i, n in enumerate(out_names)
                }
                for c in range(n_cores)
            ]

    r.fn = fn
    r.pack = pack
    r.call = call

    def run(in_maps):
        return call(pack(in_maps))

    r.run = run
    _CACHE[key] = r
    return r


def make_in_maps(inputs, cfg):
    n_cores = cfg["n_cores"]
    BL = cfg["BL"]
    weights = {
        k: np.asarray(inputs[k], np.float32)
        for k in (
            "w_fw0", "b_fw0", "w_bw0", "b_bw0",
            "w_fw1", "b_fw1", "w_bw1", "b_bw1",
            "dense_w", "dense_b", "trans",
        )
    }
    emb = np.asarray(inputs["emb"], np.float32)
    lens = np.asarray(inputs["seq_lens"], np.int64)
    tgt = np.asarray(inputs["targets"], np.int64)
    in_maps = []
    for c in range(n_cores):
        sl = slice(c * BL, (c + 1) * BL)
        in_maps.append(_prep_core(emb[sl], lens[sl], tgt[sl], weights, cfg))
    return in_maps


def kernel(**inputs):
    cfg = dict(T=T, BL=B // N_CORES, E=E, H=H, K=K, n_cores=N_CORES)
    in_maps = make_in_maps(inputs, cfg)
    runner = _get_runner(cfg)
    res = runner.run(in_maps)
    total = sum(float(r["out"][0, 0]) for r in res)
    return np.asarray(np.float32(total / B))
